# revision 1
# baseline (speedup 1.0000x reference)
"""Trainium2 8-core kernel for nn_ACCSLP_59485297050024.

The reference is a multiplicative-update NMF-style solver on N=4096 nodes with
rank R=128 and N_ITERS=2, returning a scalar objective O.

Because U, H, W, V are initialized to all-ones (per the problem's input spec),
every multiplicative update keeps each factor CONSTANT along the rank axis, so
the whole computation collapses exactly to rank-1 vector recurrences:

    u1 = (rowsum(S) + b*rowsum(Z)) * 2/(3R)
    h1 = (S + a*X)^T (1/e1) / R,  e1 = u1 + a       v1 = Z^T (1/u1) / R
    w1 = X (1/h1) / R,   u2 = (S + b*Z)(1/d1) / R,  d1 = h1 + b*v1
    h2 = (S + a*X)^T (1/e2) / R,  e2 = u2 + a*w1    v2 = Z^T (1/u2) / R
    w2 = X (1/h2) / R
    O  = R[Su2 Sh2 + a Sw2 Sh2 + b Su2 Sv2]
         - (sum(S) + a sum(X) + b sum(Z)) log R
         - <log u2, rsS + b rsZ> - a <log w2, rsX>
         - <log h2, csS + a csX> - b <log v2, csZ>

S only ever appears combined: P = S + a*X (h updates) and Q = S + b*Z
(u updates), so the device streams FOUR matrices (P, Z row-major; Q, X
col-major) -- in FP8 E4M3 (validated: objective rel err ~6e-4 vs f32 ref).

Device strategy (8 NeuronCores): row-shard (512 rows/core); resident fp8
shards in SBUF. Every contraction is a TensorE DoubleRow fp8 matmul (2
partition-tiles contracted per pass => 2x bf16 rate) with a tiny stationary
vector. The reciprocal stationaries (~5e-4) would quantize terribly in fp8,
so each is stored as a centered delta: y = c + dy/16 with a compile-time
center c; the c*colsum / c*rowsum term is reconstructed from host-computed
input statistics (csP/csZ/rsx/u1pre), which also removes the colsum columns
and one AllReduce entirely. Cross-core reduction: AR1 (h1/v1 deltas, 32KB)
and AR2 (h2 deltas, 16KB); a dummy AllReduce issued first absorbs the
per-execution collective bring-up under the DMA load phase. Final scalar
assembly runs on host in float64.
"""

import numpy as np
import ml_dtypes

N = 4096
R = 128
ALPHA = 0.5
BETA = 0.5
N_CORES = 8
RPC = N // N_CORES          # rows per core = 512
RG = RPC // 128             # row groups per core = 4
NC_CH = N // 128            # 128-column chunks = 32
NJ8 = N // 512              # 512-column chunks = 8

F = 16.0                    # delta-stationary scale
# centered-delta constants (binary-exact in f32); y_scaled = c + dy/F
C1P = 0.96875               # y_s = 2^11/(R(u1+a))
C1Z = 1.0                   # y_s = 2^11/(R u1)
CQ = 1.015625               # y_s = 2^8/(R(h1+b v1))
CX = 1.359375               # y_s = 2^8/(R h1)
C2P = 0.921875              # y_s = 2^11/(R(u2+a w1))
C2Z = 1.328125              # y_s = 2^11/(R u2)
C3 = 1.421875               # y_s = 2^8/(R h2)

_CACHED = {}
USE_DUMMY_AR = False


def _build():
    import concourse.mybir as mybir
    import concourse.tile as tile
    from concourse import bacc
    from concourse.masks import make_identity

    f8 = mybir.dt.float8e4
    f32 = mybir.dt.float32
    DR = mybir.MatmulPerfMode.DoubleRow

    nc = bacc.Bacc("TRN2", target_bir_lowering=False, debug=False,
                   num_devices=N_CORES, dynamic_dma_scratch_size=8192)

    # per-core external inputs (host supplies per-partition-contiguous layouts)
    rp_e = nc.declare_dram_parameter("rp", [128, RG, N], f8, isOutput=False)
    rz_e = nc.declare_dram_parameter("rz", [128, RG, N], f8, isOutput=False)
    cq_e = nc.declare_dram_parameter("cq", [128, NC_CH, RPC], f8, isOutput=False)
    cx_e = nc.declare_dram_parameter("cx", [128, NC_CH, RPC], f8, isOutput=False)
    s1p_e = nc.declare_dram_parameter("s1p", [128, NJ8, 2, 2, 32], f8, isOutput=False)
    s1z_e = nc.declare_dram_parameter("s1z", [128, NJ8, 2, 2, 32], f8, isOutput=False)
    u1pre_e = nc.declare_dram_parameter("u1pre", [1, RPC], f32, isOutput=False)
    rsx_e = nc.declare_dram_parameter("rsx", [1, RPC], f32, isOutput=False)
    csp_e = nc.declare_dram_parameter("csp", [NC_CH, 128], f32, isOutput=False)
    csz_e = nc.declare_dram_parameter("csz", [NC_CH, 128], f32, isOutput=False)
    # per-core external outputs
    out_u2 = nc.declare_dram_parameter("u2f", [1, RPC], f32, isOutput=True)
    out_w2 = nc.declare_dram_parameter("w2f", [1, RPC], f32, isOutput=True)
    out_vp = nc.declare_dram_parameter("vp", [NC_CH, 128], f32, isOutput=True)
    out_h2 = nc.declare_dram_parameter("h2f", [NC_CH, 128], f32, isOutput=True)

    ar1_out = nc.dram_tensor("ar1_out", [2, NC_CH, 128], f32, addr_space="Shared")
    ar2_out = nc.dram_tensor("ar2_out", [1, NC_CH, 128], f32, addr_space="Shared")
    groups = [list(range(N_CORES))]

    with tile.TileContext(nc) as tc:
        with (
            tc.tile_pool(name="big", bufs=1) as big,
            tc.tile_pool(name="small", bufs=1) as small,
            tc.tile_pool(name="stg", bufs=2) as stg,
            tc.tile_pool(name="pacc", bufs=1, space="PSUM") as pacc,
            tc.tile_pool(name="pstream", bufs=2, space="PSUM") as pstream,
            tc.tile_pool(name="ptrans", bufs=1, space="PSUM") as ptrans,
            tc.tile_pool(name="dram", bufs=1, space="DRAM") as dram,
        ):
            # ---------- dummy AllReduce first ----------
            # absorbs per-execution collective bring-up + core start skew
            # while the DMA loads run. Input is never written (garbage) and
            # output never read -- must not wait on anything.
            if USE_DUMMY_AR:
                sync_in = nc.dram_tensor("sync_in", [1, 128], f32)
                sync_out = nc.dram_tensor("sync_out", [1, 128], f32, addr_space="Shared")
                with tc.high_priority():
                    nc.gpsimd.collective_compute(
                        "AllReduce", mybir.AluOpType.add, replica_groups=groups,
                        ins=[sync_in[:].opt()], outs=[sync_out[:].opt()])

            # ---------- small inputs ----------
            s1p = small.tile([128, NJ8, 2, 2, 32], f8, tag="s1p")
            s1z = small.tile([128, NJ8, 2, 2, 32], f8, tag="s1z")
            u1pre = small.tile([1, RPC], f32, tag="u1pre")
            rsx = small.tile([1, RPC], f32, tag="rsx")
            csp = small.tile([NC_CH, 128], f32, tag="csp")
            csz = small.tile([NC_CH, 128], f32, tag="csz")
            for t, e in ((s1p, s1p_e), (s1z, s1z_e), (u1pre, u1pre_e),
                         (rsx, rsx_e), (csp, csp_e), (csz, csz_e)):
                nc.sync.dma_start(t[:], e[:])

            ident = small.tile([128, 128], f32, tag="ident")
            make_identity(nc, ident[:])

            # 64-wide interleaved-reversed fp8 stationaries (dy pair lands at
            # flat cols 62/63 = hw column 0); zero-fill once, off critical path
            statQ = small.tile([128, NC_CH // 2, 2, 32], f8, tag="statQ")
            statX = small.tile([128, NC_CH // 2, 2, 32], f8, tag="statX")
            statP2 = small.tile([128, NJ8, 2, 2, 32], f8, tag="statP2")
            statZ2 = small.tile([128, NJ8, 2, 2, 32], f8, tag="statZ2")
            stat3 = small.tile([128, NC_CH // 2, 2, 32], f8, tag="stat3")
            for t in (statQ, statX, statP2, statZ2, stat3):
                nc.gpsimd.memset(t[:], 0.0)

            # ---------- resident loads (pieces, ordered for pipelining) ----
            QC = 4   # col-tile pieces per matrix (8 chunks each)
            QR = 4   # row-tile pieces per matrix (1024 cols each)
            W4 = N // QR
            tRp4 = [big.tile([128, RG, W4], f8, name=f"tR_p{q}", tag=f"tR_p{q}") for q in range(QR)]
            tRz4 = [big.tile([128, RG, W4], f8, name=f"tR_z{q}", tag=f"tR_z{q}") for q in range(QR)]
            tCq4 = [big.tile([128, 8, RPC], f8, name=f"tC_q{q}", tag=f"tC_q{q}") for q in range(QC)]
            tCx4 = [big.tile([128, 8, RPC], f8, name=f"tC_x{q}", tag=f"tC_x{q}") for q in range(QC)]

            # rows of P/Z first (B1 is the critical path), then cols Q, X
            for q in range(QR):
                nc.sync.dma_start(tRp4[q][:], rp_e[:, :, q * W4:(q + 1) * W4])
                nc.sync.dma_start(tRz4[q][:], rz_e[:, :, q * W4:(q + 1) * W4])
            for q in range(QC):
                nc.sync.dma_start(tCq4[q][:], cq_e[:, q * 8:(q + 1) * 8, :])
            for q in range(QC):
                nc.sync.dma_start(tCx4[q][:], cx_e[:, q * 8:(q + 1) * 8, :])

            def tR(pieces, c8):
                """moving AP [128, 2, 512] pairs for chunk c8, yielded per a-pair"""
                w = (c8 % 2) * RPC
                piece = pieces[c8 // 2]
                return [piece[:, a:a + 2, w:w + RPC] for a in (0, 2)]

            def tC(pieces, c):
                return pieces[c // 8][:, (c % 8):(c % 8) + 2, :]

            # ---------- B1: dy-contractions over rows of P and Z ----------
            ar1_in = dram.tile([2, NC_CH, 128], f32, tag="ar1_in")
            psP = pstream.tile([32, RPC], f32, tag="bpsP")
            psZ = pstream.tile([32, RPC], f32, tag="bpsZ")
            for c8 in range(NJ8):
                movP = tR(tRp4, c8)
                movZ = tR(tRz4, c8)
                for i in range(2):
                    nc.tensor.matmul(psP[:], s1p[:, c8, i, :, :], movP[i],
                                     start=(c8 == 0 and i == 0),
                                     stop=(c8 == NJ8 - 1 and i == 1), perf_mode=DR)
                for i in range(2):
                    nc.tensor.matmul(psZ[:], s1z[:, c8, i, :, :], movZ[i],
                                     start=(c8 == 0 and i == 0),
                                     stop=(c8 == NJ8 - 1 and i == 1), perf_mode=DR)
            stP = stg.tile([NJ8, RPC], f32, tag="stP")
            stZ = stg.tile([NJ8, RPC], f32, tag="stZ")
            nc.scalar.copy(stP[:], psP[0:NJ8, :])
            nc.vector.tensor_copy(stZ[:], psZ[0:NJ8, :])
            nc.gpsimd.dma_start(ar1_in[0], stP[:])
            nc.gpsimd.dma_start(ar1_in[1], stZ[:])
            nc.gpsimd.collective_compute(
                "AllReduce", mybir.AluOpType.add, replica_groups=groups,
                ins=[ar1_in.opt()], outs=[ar1_out[:].opt()])

            # ---------- A2 stationaries from AR1 ----------
            # ar1 rows: 0 = sum P^T dy1P, 1 = sum Z^T dy1Z
            dP = small.tile([NC_CH, 128], f32, tag="dP")
            dZ = small.tile([NC_CH, 128], f32, tag="dZ")
            nc.gpsimd.dma_start(dP[:], ar1_out[0])
            nc.gpsimd.dma_start(dZ[:], ar1_out[1])
            h1F = small.tile([NC_CH, 128], f32, tag="h1F")    # F * 2^11 h1
            v1F = small.tile([NC_CH, 128], f32, tag="v1F")
            tQ2 = small.tile([NC_CH, 128], f32, tag="tQ2")
            nc.vector.scalar_tensor_tensor(h1F[:], csp[:], C1P * F, dP[:],
                                           mybir.AluOpType.mult, mybir.AluOpType.add)
            nc.vector.scalar_tensor_tensor(v1F[:], csz[:], C1Z * F, dZ[:],
                                           mybir.AluOpType.mult, mybir.AluOpType.add)
            nc.vector.scalar_tensor_tensor(tQ2[:], v1F[:], BETA, h1F[:],
                                           mybir.AluOpType.mult, mybir.AluOpType.add)
            ps_t2 = ptrans.tile([128, 96], f32, tag="pt")
            nc.tensor.transpose(ps_t2[:, 0:NC_CH], tQ2[:], ident[0:NC_CH, 0:NC_CH])
            nc.tensor.transpose(ps_t2[:, NC_CH:2 * NC_CH], h1F[:],
                                ident[0:NC_CH, 0:NC_CH])
            pre_a = small.tile([128, 2 * NC_CH], f32, tag="pre_a")
            nc.vector.reciprocal(pre_a[:], ps_t2[:, 0:2 * NC_CH])
            nc.vector.tensor_scalar(
                statQ[:, :, :, 0],
                pre_a[:, 0:NC_CH].rearrange("p (c j) -> p c j", j=2),
                float(2 ** 20), -F * CQ,
                mybir.AluOpType.mult, mybir.AluOpType.add)
            nc.vector.tensor_scalar(
                statX[:, :, :, 0],
                pre_a[:, NC_CH:2 * NC_CH].rearrange("p (c j) -> p c j", j=2),
                float(2 ** 20), -F * CX,
                mybir.AluOpType.mult, mybir.AluOpType.add)

            # ---------- A2 streams: u2, w1 ----------
            psQ = pacc.tile([32, RPC], f32, tag="accQ")
            for i, c in enumerate(range(0, NC_CH, 2)):
                nc.tensor.matmul(psQ[:], statQ[:, i, :, :], tC(tCq4, c),
                                 start=(i == 0), stop=(i == 15), perf_mode=DR)
            psX = pacc.tile([32, RPC], f32, tag="accX")
            for i, c in enumerate(range(0, NC_CH, 2)):
                nc.tensor.matmul(psX[:], statX[:, i, :, :], tC(tCx4, c),
                                 start=(i == 0), stop=(i == 15), perf_mode=DR)
            u2F = small.tile([1, RPC], f32, tag="u2F")
            w1F = small.tile([1, RPC], f32, tag="w1F")
            t2F = small.tile([1, RPC], f32, tag="t2F")
            nc.vector.scalar_tensor_tensor(u2F[:], u1pre[:], CQ * F, psQ[0:1, :],
                                           mybir.AluOpType.mult, mybir.AluOpType.add)
            nc.vector.scalar_tensor_tensor(w1F[:], rsx[:], CX * F, psX[0:1, :],
                                           mybir.AluOpType.mult, mybir.AluOpType.add)
            nc.vector.scalar_tensor_tensor(t2F[:], w1F[:], ALPHA, u2F[:],
                                           mybir.AluOpType.mult, mybir.AluOpType.add)
            nc.sync.dma_start(out_u2[:], u2F[:])

            # ---------- B2 stationaries ----------
            ps_t3 = ptrans.tile([128, 96], f32, tag="pt")
            for v, yv in enumerate((t2F, u2F)):
                for a in range(RG):
                    nc.tensor.transpose(
                        ps_t3[:, v * RG + a: v * RG + a + 1],
                        yv[0:1, a * 128:(a + 1) * 128],
                        ident[0:1, 0:1])
            pre_b = small.tile([128, 2 * RG], f32, tag="pre_b")
            nc.vector.reciprocal(pre_b[:], ps_t3[:, 0:2 * RG])
            for c8 in range(NJ8):
                nc.vector.tensor_scalar(
                    statP2[:, c8, :, :, c8],
                    pre_b[:, 0:RG].rearrange("p (c j) -> p c j", j=2),
                    float(2 ** 20), -F * C2P,
                    mybir.AluOpType.mult, mybir.AluOpType.add)
                nc.vector.tensor_scalar(
                    statZ2[:, c8, :, :, c8],
                    pre_b[:, RG:2 * RG].rearrange("p (c j) -> p c j", j=2),
                    float(2 ** 20), -F * C2Z,
                    mybir.AluOpType.mult, mybir.AluOpType.add)

            # ---------- B2 streams ----------
            # P (h2) first: its AllReduce is on the critical path; the Z (v2)
            # stream overlaps the AR2 wait.
            ar2_in = dram.tile([1, NC_CH, 128], f32, tag="ar2_in")
            psP2 = pstream.tile([32, RPC], f32, tag="bpsP")
            for c8 in range(NJ8):
                movP = tR(tRp4, c8)
                for i in range(2):
                    nc.tensor.matmul(psP2[:], statP2[:, c8, i, :, :], movP[i],
                                     start=(c8 == 0 and i == 0),
                                     stop=(c8 == NJ8 - 1 and i == 1), perf_mode=DR)
            stP2 = stg.tile([NJ8, RPC], f32, tag="stP")
            nc.scalar.copy(stP2[:], psP2[0:NJ8, :])
            nc.gpsimd.dma_start(ar2_in[0], stP2[:])
            nc.gpsimd.collective_compute(
                "AllReduce", mybir.AluOpType.add, replica_groups=groups,
                ins=[ar2_in.opt()], outs=[ar2_out[:].opt()])
            psZ2 = pstream.tile([32, RPC], f32, tag="bpsZ")
            for c8 in range(NJ8):
                movZ = tR(tRz4, c8)
                for i in range(2):
                    nc.tensor.matmul(psZ2[:], statZ2[:, c8, i, :, :], movZ[i],
                                     start=(c8 == 0 and i == 0),
                                     stop=(c8 == NJ8 - 1 and i == 1), perf_mode=DR)
            stZ2 = stg.tile([NJ8, RPC], f32, tag="stZ")
            nc.vector.tensor_copy(stZ2[:], psZ2[0:NJ8, :])
            nc.sync.dma_start(out_vp[:], stZ2[:])

            # ---------- A3: w2 ----------
            d2 = small.tile([NC_CH, 128], f32, tag="d2")
            nc.gpsimd.dma_start(d2[:], ar2_out[0])
            h2F = small.tile([NC_CH, 128], f32, tag="h2F")
            nc.vector.scalar_tensor_tensor(h2F[:], csp[:], C2P * F, d2[:],
                                           mybir.AluOpType.mult, mybir.AluOpType.add)
            nc.sync.dma_start(out_h2[:], h2F[:])
            ps_t4 = ptrans.tile([128, 96], f32, tag="pt")
            nc.tensor.transpose(ps_t4[:, 0:NC_CH], h2F[:], ident[0:NC_CH, 0:NC_CH])
            pre_3 = small.tile([128, NC_CH], f32, tag="pre_3")
            nc.vector.reciprocal(pre_3[:], ps_t4[:, 0:NC_CH])
            nc.vector.tensor_scalar(
                stat3[:, :, :, 0],
                pre_3[:].rearrange("p (c j) -> p c j", j=2),
                float(2 ** 20), -F * C3,
                mybir.AluOpType.mult, mybir.AluOpType.add)
            ps_a3 = pacc.tile([32, RPC], f32, tag="accQ")
            for i, c in enumerate(range(0, NC_CH, 2)):
                nc.tensor.matmul(ps_a3[:], stat3[:, i, :, :], tC(tCx4, c),
                                 start=(i == 0), stop=(i == 15), perf_mode=DR)
            w2F = small.tile([1, RPC], f32, tag="w2F")
            nc.vector.scalar_tensor_tensor(w2F[:], rsx[:], C3 * F, ps_a3[0:1, :],
                                           mybir.AluOpType.mult, mybir.AluOpType.add)
            nc.sync.dma_start(out_w2[:], w2F[:])

    nc.compile()
    return nc


def _host_stats(S, Z, X):
    """fp8 casts + input statistics; returns per-core in_maps and host data."""
    S = np.asarray(S, np.float32)
    Z = np.asarray(Z, np.float32)
    X = np.asarray(X, np.float32)
    P8 = (S + ALPHA * X).astype(ml_dtypes.float8_e4m3)
    Q8 = (S + BETA * Z).astype(ml_dtypes.float8_e4m3)
    X8 = X.astype(ml_dtypes.float8_e4m3)
    Z8 = Z.astype(ml_dtypes.float8_e4m3)

    Pf = P8.astype(np.float32)
    Qf = Q8.astype(np.float32)
    Xf = X8.astype(np.float32)
    Zf = Z8.astype(np.float32)
    u1pre = Qf.sum(axis=1)                  # rowsum(Q)  (N,)
    rsx = Xf.sum(axis=1)                    # rowsum(X)
    csp = Pf.sum(axis=0)                    # colsum(P)  (N,)
    csz = Zf.sum(axis=0)

    qq = u1pre * (2.0 / 3.0)                # R*u1
    dy1p = ((2048.0 / (qq + 64.0)) - C1P) * F
    dy1z = ((2048.0 / qq) - C1Z) * F

    def row_layout(shard):
        # [512, 4096] -> [128(p), RG(a), 4096(j)], per-partition contiguous
        return np.ascontiguousarray(shard.reshape(RG, 128, N).transpose(1, 0, 2))

    def col_layout(shard):
        # [512, 4096] -> [128(p), NC_CH(c), 512(l)] where (c,p) indexes col j
        return np.ascontiguousarray(
            shard.T.reshape(NC_CH, 128, RPC).transpose(1, 0, 2))

    def vec_layout(v):
        # [512] -> [128(p), NJ8(c8), 2(pair), 2(k), 32(col)]: DoubleRow
        # ldweights [k=2, cols] planes; dy sits in col c8 so chunk c8's
        # output lands on PSUM row c8 (one wide staging copy per stream)
        g = v.reshape(RG, 128).T                 # [128, RG]
        out = np.zeros((128, NJ8, 2, 2, 32), v.dtype)
        for c8 in range(NJ8):
            out[:, c8, :, 0, c8] = g[:, 0::2]
            out[:, c8, :, 1, c8] = g[:, 1::2]
        return out

    csp_t = np.ascontiguousarray(csp.reshape(NC_CH, 128))
    csz_t = np.ascontiguousarray(csz.reshape(NC_CH, 128))

    in_maps = []
    for c in range(N_CORES):
        rows = slice(c * RPC, (c + 1) * RPC)
        in_maps.append({
            "rp": row_layout(P8[rows]), "rz": row_layout(Z8[rows]),
            "cq": col_layout(Q8[rows]), "cx": col_layout(X8[rows]),
            "s1p": vec_layout(dy1p[rows].astype(ml_dtypes.float8_e4m3)),
            "s1z": vec_layout(dy1z[rows].astype(ml_dtypes.float8_e4m3)),
            "u1pre": u1pre[rows].reshape(1, RPC).astype(np.float32),
            "rsx": rsx[rows].reshape(1, RPC).astype(np.float32),
            "csp": csp_t, "csz": csz_t,
        })
    host = {"u1pre": u1pre, "rsx": rsx, "csp": csp, "csz": csz}
    return in_maps, host


def _make_in_maps(S, Z, X):
    in_maps, host = _host_stats(S, Z, X)
    _CACHED["host"] = host
    return in_maps


def _finale(res):
    """Assemble the scalar objective from device outputs (float64)."""
    host = _CACHED["host"]
    u1pre = host["u1pre"].astype(np.float64)
    rsx = host["rsx"].astype(np.float64)
    csp = host["csp"].astype(np.float64)
    csz = host["csz"].astype(np.float64)

    u2F = np.concatenate([np.asarray(res[i]["u2f"], np.float64).ravel()
                          for i in range(N_CORES)])
    w2F = np.concatenate([np.asarray(res[i]["w2f"], np.float64).ravel()
                          for i in range(N_CORES)])
    h2F = np.asarray(res[0]["h2f"], np.float64).ravel()
    vp = np.sum([np.asarray(res[i]["vp"], np.float64).ravel()
                 for i in range(N_CORES)], axis=0)

    u2 = u2F / (F * 2 ** 8)
    w2 = w2F / (F * 2 ** 8)
    h2 = h2F / (F * 2 ** 11)
    v2 = (C2Z * csz + vp / F) / 2 ** 11

    rs_sz = u1pre                       # = rsS + b*rsZ
    lR = np.log(R)
    term1 = R * (u2.sum() * h2.sum() + ALPHA * w2.sum() * h2.sum()
                 + BETA * u2.sum() * v2.sum())
    O = (term1
         - (csp.sum() + BETA * csz.sum()) * lR
         - (np.log(u2) * rs_sz).sum()
         - ALPHA * (np.log(w2) * rsx).sum()
         - (np.log(h2) * csp).sum()
         - BETA * (np.log(v2) * csz).sum())
    return np.float32(O)


def _numpy_fallback(S, Z, X, U, H, W, V):
    """Faithful CPU implementation (only used if factors are not all-ones)."""
    S, Z, X, U, H, W, V = [np.asarray(a, np.float32) for a in (S, Z, X, U, H, W, V)]

    def obj(Sp, Xp, Zp):
        return ((Sp - S * np.log(Sp)).sum()
                + ALPHA * (Xp - X * np.log(Xp)).sum()
                + BETA * (Zp - Z * np.log(Zp)).sum())

    Sp = U @ H; Xp = W @ H; Zp = U @ V
    Sd = S / Sp; Xd = X / Xp; Zd = Z / Zp
    O = obj(Sp, Xp, Zp)
    for _ in range(2):
        dHV = H + BETA * V
        U = U * (Sd @ (H / dHV).T + Zd @ ((BETA * V) / dHV).T)
        Sp = U @ H; Zp = U @ V; Sd = S / Sp; Zd = Z / Zp
        dUW = U + ALPHA * W
        H = H * ((U / dUW).T @ Sd + ((ALPHA * W) / dUW).T @ Xd)
        Sp = U @ H; Xp = W @ H; Sd = S / Sp; Xd = X / Xp
        W = W * Xd.sum(axis=1, keepdims=True)
        Xp = W @ H; Xd = X / Xp
        V = V * Zd.sum(axis=0, keepdims=True)
        Zp = U @ V; Zd = Z / Zp
        O = obj(Sp, Xp, Zp)
    return np.float32(O)


def kernel(S, Z, X, U, H, W, V):
    if not (np.all(np.asarray(U) == 1) and np.all(np.asarray(H) == 1)
            and np.all(np.asarray(W) == 1) and np.all(np.asarray(V) == 1)):
        return _numpy_fallback(S, Z, X, U, H, W, V)

    import time
    from concourse.bass_utils import run_bass_kernel_spmd

    if "nc" not in _CACHED:
        _CACHED["nc"] = _build()
    nc = _CACHED["nc"]

    in_maps = _make_in_maps(S, Z, X)
    last = None
    for attempt in range(3):
        try:
            res = run_bass_kernel_spmd(nc, in_maps, core_ids=list(range(N_CORES)))
            return _finale(res.results)
        except Exception as e:  # transient NRT/device errors: reset and retry
            last = e
            try:
                import jax
                jax.clear_caches()
                jax.clear_backends()
            except Exception:
                pass
            time.sleep(3.0)
    raise last


if __name__ == "__main__":
    import reference
    inputs = reference.setup_inputs()
    inputs = {k: np.asarray(v) for k, v in inputs.items()}
    print("kernel:", kernel(**inputs))



# revision 2
# speedup vs baseline: 1.0960x; 1.0960x over previous
"""Trainium2 8-core kernel for nn_ACCSLP_59485297050024.

The reference is a multiplicative-update NMF-style solver on N=4096 nodes with
rank R=128 and N_ITERS=2, returning a scalar objective O.

Because U, H, W, V are initialized to all-ones (per the problem's input spec),
every multiplicative update keeps each factor CONSTANT along the rank axis, so
the whole computation collapses exactly to rank-1 vector recurrences:

    u1 = (rowsum(S) + b*rowsum(Z)) * 2/(3R)
    h1 = (S + a*X)^T (1/e1) / R,  e1 = u1 + a       v1 = Z^T (1/u1) / R
    w1 = X (1/h1) / R,   u2 = (S + b*Z)(1/d1) / R,  d1 = h1 + b*v1
    h2 = (S + a*X)^T (1/e2) / R,  e2 = u2 + a*w1    v2 = Z^T (1/u2) / R
    w2 = X (1/h2) / R
    O  = R[Su2 Sh2 + a Sw2 Sh2 + b Su2 Sv2]
         - (sum(S) + a sum(X) + b sum(Z)) log R
         - <log u2, rsS + b rsZ> - a <log w2, rsX>
         - <log h2, csS + a csX> - b <log v2, csZ>

S only ever appears combined: P = S + a*X (h updates) and Q = S + b*Z
(u updates), so the device streams FOUR matrices (P, Z row-major; Q, X
col-major) -- in FP8 E4M3 (validated: objective rel err ~6e-4 vs f32 ref).

Device strategy (8 NeuronCores): row-shard (512 rows/core); resident fp8
shards in SBUF. Every contraction is a TensorE DoubleRow fp8 matmul (2
partition-tiles contracted per pass => 2x bf16 rate) with a tiny stationary
vector. The reciprocal stationaries (~5e-4) would quantize terribly in fp8,
so each is stored as a centered delta: y = c + dy/16 with a compile-time
center c; the c*colsum / c*rowsum term is reconstructed from host-computed
input statistics (csP/csZ/rsx/u1pre), which also removes the colsum columns
and one AllReduce entirely. Cross-core reduction: AR1 (h1/v1 deltas, 32KB)
and AR2 (h2 deltas, 16KB); a dummy AllReduce issued first absorbs the
per-execution collective bring-up under the DMA load phase. Final scalar
assembly runs on host in float64.
"""

import numpy as np
import ml_dtypes

N = 4096
R = 128
ALPHA = 0.5
BETA = 0.5
N_CORES = 8
RPC = N // N_CORES          # rows per core = 512
RG = RPC // 128             # row groups per core = 4
NC_CH = N // 128            # 128-column chunks = 32
NJ8 = N // 512              # 512-column chunks = 8

F = 16.0                    # delta-stationary scale
# centered-delta constants (binary-exact in f32); y_scaled = c + dy/F
C1P = 0.96875               # y_s = 2^11/(R(u1+a))
C1Z = 1.0                   # y_s = 2^11/(R u1)
CQ = 1.015625               # y_s = 2^8/(R(h1+b v1))
CX = 1.359375               # y_s = 2^8/(R h1)
C2P = 0.921875              # y_s = 2^11/(R(u2+a w1))
C2Z = 1.328125              # y_s = 2^11/(R u2)
C3 = 1.421875               # y_s = 2^8/(R h2)

_CACHED = {}
USE_DUMMY_AR = True


def _build():
    import concourse.mybir as mybir
    import concourse.tile as tile
    from concourse import bacc
    from concourse.masks import make_identity

    f8 = mybir.dt.float8e4
    f32 = mybir.dt.float32
    DR = mybir.MatmulPerfMode.DoubleRow

    nc = bacc.Bacc("TRN2", target_bir_lowering=False, debug=False,
                   num_devices=N_CORES, dynamic_dma_scratch_size=8192)

    # per-core external inputs (host supplies per-partition-contiguous layouts)
    rp_e = nc.declare_dram_parameter("rp", [128, RG, N], f8, isOutput=False)
    rz_e = nc.declare_dram_parameter("rz", [128, RG, N], f8, isOutput=False)
    cq_e = nc.declare_dram_parameter("cq", [128, NC_CH, RPC], f8, isOutput=False)
    cx_e = nc.declare_dram_parameter("cx", [128, NC_CH, RPC], f8, isOutput=False)
    s1p_e = nc.declare_dram_parameter("s1p", [128, NJ8, 2, 2, 32], f8, isOutput=False)
    s1z_e = nc.declare_dram_parameter("s1z", [128, NJ8, 2, 2, 32], f8, isOutput=False)
    u1pre_e = nc.declare_dram_parameter("u1pre", [1, RPC], f32, isOutput=False)
    rsx_e = nc.declare_dram_parameter("rsx", [1, RPC], f32, isOutput=False)
    csp_e = nc.declare_dram_parameter("csp", [NC_CH, 128], f32, isOutput=False)
    csz_e = nc.declare_dram_parameter("csz", [NC_CH, 128], f32, isOutput=False)
    # per-core external outputs
    out_u2 = nc.declare_dram_parameter("u2f", [1, RPC], f32, isOutput=True)
    out_w2 = nc.declare_dram_parameter("w2f", [1, RPC], f32, isOutput=True)
    out_vp = nc.declare_dram_parameter("vp", [NC_CH, 128], f32, isOutput=True)
    out_h2 = nc.declare_dram_parameter("h2f", [NC_CH, 128], f32, isOutput=True)

    ar1_out = nc.dram_tensor("ar1_out", [2, NC_CH, 128], f32, addr_space="Shared")
    ar2_out = nc.dram_tensor("ar2_out", [1, NC_CH, 128], f32, addr_space="Shared")
    groups = [list(range(N_CORES))]

    with tile.TileContext(nc) as tc:
        with (
            tc.tile_pool(name="big", bufs=1) as big,
            tc.tile_pool(name="small", bufs=1) as small,
            tc.tile_pool(name="stg", bufs=2) as stg,
            tc.tile_pool(name="pacc", bufs=1, space="PSUM") as pacc,
            tc.tile_pool(name="pstream", bufs=2, space="PSUM") as pstream,
            tc.tile_pool(name="ptrans", bufs=1, space="PSUM") as ptrans,
            tc.tile_pool(name="dram", bufs=1, space="DRAM") as dram,
        ):
            # ---------- dummy AllReduce first ----------
            # absorbs per-execution collective bring-up + core start skew
            # while the DMA loads run. Input is never written (garbage) and
            # output never read -- must not wait on anything.
            if USE_DUMMY_AR:
                sync_in = nc.dram_tensor("sync_in", [1, 128], f32)
                sync_out = nc.dram_tensor("sync_out", [1, 128], f32, addr_space="Shared")
                with tc.high_priority():
                    nc.gpsimd.collective_compute(
                        "AllReduce", mybir.AluOpType.add, replica_groups=groups,
                        ins=[sync_in[:].opt()], outs=[sync_out[:].opt()])

            # ---------- small inputs ----------
            s1p = small.tile([128, NJ8, 2, 2, 32], f8, tag="s1p")
            s1z = small.tile([128, NJ8, 2, 2, 32], f8, tag="s1z")
            u1pre = small.tile([1, RPC], f32, tag="u1pre")
            rsx = small.tile([1, RPC], f32, tag="rsx")
            csp = small.tile([NC_CH, 128], f32, tag="csp")
            csz = small.tile([NC_CH, 128], f32, tag="csz")
            for t, e in ((s1p, s1p_e), (s1z, s1z_e), (u1pre, u1pre_e),
                         (rsx, rsx_e), (csp, csp_e), (csz, csz_e)):
                nc.sync.dma_start(t[:], e[:])

            ident = small.tile([128, 128], f32, tag="ident")
            make_identity(nc, ident[:])

            # 64-wide interleaved-reversed fp8 stationaries (dy pair lands at
            # flat cols 62/63 = hw column 0); zero-fill once, off critical path
            statQ = small.tile([128, NC_CH // 2, 2, 32], f8, tag="statQ")
            statX = small.tile([128, NC_CH // 2, 2, 32], f8, tag="statX")
            statP2 = small.tile([128, NJ8, 2, 2, 32], f8, tag="statP2")
            statZ2 = small.tile([128, NJ8, 2, 2, 32], f8, tag="statZ2")
            stat3 = small.tile([128, NC_CH // 2, 2, 32], f8, tag="stat3")
            for t in (statQ, statX, statP2, statZ2, stat3):
                nc.gpsimd.memset(t[:], 0.0)

            # ---------- resident loads (pieces, ordered for pipelining) ----
            QC = 4   # col-tile pieces per matrix (8 chunks each)
            QR = 4   # row-tile pieces per matrix (1024 cols each)
            W4 = N // QR
            tRp4 = [big.tile([128, RG, W4], f8, name=f"tR_p{q}", tag=f"tR_p{q}") for q in range(QR)]
            tRz4 = [big.tile([128, RG, W4], f8, name=f"tR_z{q}", tag=f"tR_z{q}") for q in range(QR)]
            tCq4 = [big.tile([128, 8, RPC], f8, name=f"tC_q{q}", tag=f"tC_q{q}") for q in range(QC)]
            tCx4 = [big.tile([128, 8, RPC], f8, name=f"tC_x{q}", tag=f"tC_x{q}") for q in range(QC)]

            # rows of P/Z first (B1 is the critical path), then cols Q, X
            for q in range(QR):
                nc.sync.dma_start(tRp4[q][:], rp_e[:, :, q * W4:(q + 1) * W4])
                nc.sync.dma_start(tRz4[q][:], rz_e[:, :, q * W4:(q + 1) * W4])
            for q in range(QC):
                nc.sync.dma_start(tCq4[q][:], cq_e[:, q * 8:(q + 1) * 8, :])
            for q in range(QC):
                nc.sync.dma_start(tCx4[q][:], cx_e[:, q * 8:(q + 1) * 8, :])

            def tR(pieces, c8):
                """moving AP [128, 2, 512] pairs for chunk c8, yielded per a-pair"""
                w = (c8 % 2) * RPC
                piece = pieces[c8 // 2]
                return [piece[:, a:a + 2, w:w + RPC] for a in (0, 2)]

            def tC(pieces, c):
                return pieces[c // 8][:, (c % 8):(c % 8) + 2, :]

            # ---------- B1: dy-contractions over rows of P and Z ----------
            ar1_in = dram.tile([2, NC_CH, 128], f32, tag="ar1_in")
            psP = pstream.tile([32, RPC], f32, tag="bpsP")
            psZ = pstream.tile([32, RPC], f32, tag="bpsZ")
            for c8 in range(NJ8):
                movP = tR(tRp4, c8)
                movZ = tR(tRz4, c8)
                for i in range(2):
                    nc.tensor.matmul(psP[:], s1p[:, c8, i, :, :], movP[i],
                                     start=(c8 == 0 and i == 0),
                                     stop=(c8 == NJ8 - 1 and i == 1), perf_mode=DR)
                for i in range(2):
                    nc.tensor.matmul(psZ[:], s1z[:, c8, i, :, :], movZ[i],
                                     start=(c8 == 0 and i == 0),
                                     stop=(c8 == NJ8 - 1 and i == 1), perf_mode=DR)
            stP = stg.tile([NJ8, RPC], f32, tag="stP")
            stZ = stg.tile([NJ8, RPC], f32, tag="stZ")
            nc.scalar.copy(stP[:], psP[0:NJ8, :])
            nc.vector.tensor_copy(stZ[:], psZ[0:NJ8, :])
            nc.gpsimd.dma_start(ar1_in[0], stP[:])
            nc.gpsimd.dma_start(ar1_in[1], stZ[:])
            nc.gpsimd.collective_compute(
                "AllReduce", mybir.AluOpType.add, replica_groups=groups,
                ins=[ar1_in.opt()], outs=[ar1_out[:].opt()])

            # ---------- A2 stationaries from AR1 ----------
            # ar1 rows: 0 = sum P^T dy1P, 1 = sum Z^T dy1Z
            dP = small.tile([NC_CH, 128], f32, tag="dP")
            dZ = small.tile([NC_CH, 128], f32, tag="dZ")
            nc.gpsimd.dma_start(dP[:], ar1_out[0])
            nc.gpsimd.dma_start(dZ[:], ar1_out[1])
            h1F = small.tile([NC_CH, 128], f32, tag="h1F")    # F * 2^11 h1
            v1F = small.tile([NC_CH, 128], f32, tag="v1F")
            tQ2 = small.tile([NC_CH, 128], f32, tag="tQ2")
            nc.vector.scalar_tensor_tensor(h1F[:], csp[:], C1P * F, dP[:],
                                           mybir.AluOpType.mult, mybir.AluOpType.add)
            nc.vector.scalar_tensor_tensor(v1F[:], csz[:], C1Z * F, dZ[:],
                                           mybir.AluOpType.mult, mybir.AluOpType.add)
            nc.vector.scalar_tensor_tensor(tQ2[:], v1F[:], BETA, h1F[:],
                                           mybir.AluOpType.mult, mybir.AluOpType.add)
            ps_t2 = ptrans.tile([128, 96], f32, tag="pt")
            nc.tensor.transpose(ps_t2[:, 0:NC_CH], tQ2[:], ident[0:NC_CH, 0:NC_CH])
            nc.tensor.transpose(ps_t2[:, NC_CH:2 * NC_CH], h1F[:],
                                ident[0:NC_CH, 0:NC_CH])
            pre_a = small.tile([128, 2 * NC_CH], f32, tag="pre_a")
            nc.vector.reciprocal(pre_a[:], ps_t2[:, 0:2 * NC_CH])
            nc.vector.tensor_scalar(
                statQ[:, :, :, 0],
                pre_a[:, 0:NC_CH].rearrange("p (c j) -> p c j", j=2),
                float(2 ** 20), -F * CQ,
                mybir.AluOpType.mult, mybir.AluOpType.add)
            nc.vector.tensor_scalar(
                statX[:, :, :, 0],
                pre_a[:, NC_CH:2 * NC_CH].rearrange("p (c j) -> p c j", j=2),
                float(2 ** 20), -F * CX,
                mybir.AluOpType.mult, mybir.AluOpType.add)

            # ---------- A2 streams: u2, w1 ----------
            psQ = pacc.tile([32, RPC], f32, tag="accQ")
            for i, c in enumerate(range(0, NC_CH, 2)):
                nc.tensor.matmul(psQ[:], statQ[:, i, :, :], tC(tCq4, c),
                                 start=(i == 0), stop=(i == 15), perf_mode=DR)
            psX = pacc.tile([32, RPC], f32, tag="accX")
            for i, c in enumerate(range(0, NC_CH, 2)):
                nc.tensor.matmul(psX[:], statX[:, i, :, :], tC(tCx4, c),
                                 start=(i == 0), stop=(i == 15), perf_mode=DR)
            u2F = small.tile([1, RPC], f32, tag="u2F")
            w1F = small.tile([1, RPC], f32, tag="w1F")
            t2F = small.tile([1, RPC], f32, tag="t2F")
            nc.vector.scalar_tensor_tensor(u2F[:], u1pre[:], CQ * F, psQ[0:1, :],
                                           mybir.AluOpType.mult, mybir.AluOpType.add)
            nc.vector.scalar_tensor_tensor(w1F[:], rsx[:], CX * F, psX[0:1, :],
                                           mybir.AluOpType.mult, mybir.AluOpType.add)
            nc.vector.scalar_tensor_tensor(t2F[:], w1F[:], ALPHA, u2F[:],
                                           mybir.AluOpType.mult, mybir.AluOpType.add)
            nc.sync.dma_start(out_u2[:], u2F[:])

            # ---------- B2 stationaries ----------
            ps_t3 = ptrans.tile([128, 96], f32, tag="pt")
            for v, yv in enumerate((t2F, u2F)):
                for a in range(RG):
                    nc.tensor.transpose(
                        ps_t3[:, v * RG + a: v * RG + a + 1],
                        yv[0:1, a * 128:(a + 1) * 128],
                        ident[0:1, 0:1])
            pre_b = small.tile([128, 2 * RG], f32, tag="pre_b")
            nc.vector.reciprocal(pre_b[:], ps_t3[:, 0:2 * RG])
            for c8 in range(NJ8):
                nc.vector.tensor_scalar(
                    statP2[:, c8, :, :, c8],
                    pre_b[:, 0:RG].rearrange("p (c j) -> p c j", j=2),
                    float(2 ** 20), -F * C2P,
                    mybir.AluOpType.mult, mybir.AluOpType.add)
                nc.vector.tensor_scalar(
                    statZ2[:, c8, :, :, c8],
                    pre_b[:, RG:2 * RG].rearrange("p (c j) -> p c j", j=2),
                    float(2 ** 20), -F * C2Z,
                    mybir.AluOpType.mult, mybir.AluOpType.add)

            # ---------- B2 streams ----------
            # P (h2) first: its AllReduce is on the critical path; the Z (v2)
            # stream overlaps the AR2 wait.
            ar2_in = dram.tile([1, NC_CH, 128], f32, tag="ar2_in")
            psP2 = pstream.tile([32, RPC], f32, tag="bpsP")
            for c8 in range(NJ8):
                movP = tR(tRp4, c8)
                for i in range(2):
                    nc.tensor.matmul(psP2[:], statP2[:, c8, i, :, :], movP[i],
                                     start=(c8 == 0 and i == 0),
                                     stop=(c8 == NJ8 - 1 and i == 1), perf_mode=DR)
            stP2 = stg.tile([NJ8, RPC], f32, tag="stP")
            nc.scalar.copy(stP2[:], psP2[0:NJ8, :])
            nc.gpsimd.dma_start(ar2_in[0], stP2[:])
            nc.gpsimd.collective_compute(
                "AllReduce", mybir.AluOpType.add, replica_groups=groups,
                ins=[ar2_in.opt()], outs=[ar2_out[:].opt()])
            psZ2 = pstream.tile([32, RPC], f32, tag="bpsZ")
            for c8 in range(NJ8):
                movZ = tR(tRz4, c8)
                for i in range(2):
                    nc.tensor.matmul(psZ2[:], statZ2[:, c8, i, :, :], movZ[i],
                                     start=(c8 == 0 and i == 0),
                                     stop=(c8 == NJ8 - 1 and i == 1), perf_mode=DR)
            stZ2 = stg.tile([NJ8, RPC], f32, tag="stZ")
            nc.vector.tensor_copy(stZ2[:], psZ2[0:NJ8, :])
            nc.sync.dma_start(out_vp[:], stZ2[:])

            # ---------- A3: w2 ----------
            d2 = small.tile([NC_CH, 128], f32, tag="d2")
            nc.gpsimd.dma_start(d2[:], ar2_out[0])
            h2F = small.tile([NC_CH, 128], f32, tag="h2F")
            nc.vector.scalar_tensor_tensor(h2F[:], csp[:], C2P * F, d2[:],
                                           mybir.AluOpType.mult, mybir.AluOpType.add)
            nc.sync.dma_start(out_h2[:], h2F[:])
            ps_t4 = ptrans.tile([128, 96], f32, tag="pt")
            nc.tensor.transpose(ps_t4[:, 0:NC_CH], h2F[:], ident[0:NC_CH, 0:NC_CH])
            pre_3 = small.tile([128, NC_CH], f32, tag="pre_3")
            nc.vector.reciprocal(pre_3[:], ps_t4[:, 0:NC_CH])
            nc.vector.tensor_scalar(
                stat3[:, :, :, 0],
                pre_3[:].rearrange("p (c j) -> p c j", j=2),
                float(2 ** 20), -F * C3,
                mybir.AluOpType.mult, mybir.AluOpType.add)
            ps_a3 = pacc.tile([32, RPC], f32, tag="accQ")
            for i, c in enumerate(range(0, NC_CH, 2)):
                nc.tensor.matmul(ps_a3[:], stat3[:, i, :, :], tC(tCx4, c),
                                 start=(i == 0), stop=(i == 15), perf_mode=DR)
            w2F = small.tile([1, RPC], f32, tag="w2F")
            nc.vector.scalar_tensor_tensor(w2F[:], rsx[:], C3 * F, ps_a3[0:1, :],
                                           mybir.AluOpType.mult, mybir.AluOpType.add)
            nc.sync.dma_start(out_w2[:], w2F[:])

    nc.compile()
    return nc


def _host_stats(S, Z, X):
    """fp8 casts + input statistics; returns per-core in_maps and host data."""
    S = np.asarray(S, np.float32)
    Z = np.asarray(Z, np.float32)
    X = np.asarray(X, np.float32)
    P8 = (S + ALPHA * X).astype(ml_dtypes.float8_e4m3)
    Q8 = (S + BETA * Z).astype(ml_dtypes.float8_e4m3)
    X8 = X.astype(ml_dtypes.float8_e4m3)
    Z8 = Z.astype(ml_dtypes.float8_e4m3)

    Pf = P8.astype(np.float32)
    Qf = Q8.astype(np.float32)
    Xf = X8.astype(np.float32)
    Zf = Z8.astype(np.float32)
    u1pre = Qf.sum(axis=1)                  # rowsum(Q)  (N,)
    rsx = Xf.sum(axis=1)                    # rowsum(X)
    csp = Pf.sum(axis=0)                    # colsum(P)  (N,)
    csz = Zf.sum(axis=0)

    qq = u1pre * (2.0 / 3.0)                # R*u1
    dy1p = ((2048.0 / (qq + 64.0)) - C1P) * F
    dy1z = ((2048.0 / qq) - C1Z) * F

    def row_layout(shard):
        # [512, 4096] -> [128(p), RG(a), 4096(j)], per-partition contiguous
        return np.ascontiguousarray(shard.reshape(RG, 128, N).transpose(1, 0, 2))

    def col_layout(shard):
        # [512, 4096] -> [128(p), NC_CH(c), 512(l)] where (c,p) indexes col j
        return np.ascontiguousarray(
            shard.T.reshape(NC_CH, 128, RPC).transpose(1, 0, 2))

    def vec_layout(v):
        # [512] -> [128(p), NJ8(c8), 2(pair), 2(k), 32(col)]: DoubleRow
        # ldweights [k=2, cols] planes; dy sits in col c8 so chunk c8's
        # output lands on PSUM row c8 (one wide staging copy per stream)
        g = v.reshape(RG, 128).T                 # [128, RG]
        out = np.zeros((128, NJ8, 2, 2, 32), v.dtype)
        for c8 in range(NJ8):
            out[:, c8, :, 0, c8] = g[:, 0::2]
            out[:, c8, :, 1, c8] = g[:, 1::2]
        return out

    csp_t = np.ascontiguousarray(csp.reshape(NC_CH, 128))
    csz_t = np.ascontiguousarray(csz.reshape(NC_CH, 128))

    in_maps = []
    for c in range(N_CORES):
        rows = slice(c * RPC, (c + 1) * RPC)
        in_maps.append({
            "rp": row_layout(P8[rows]), "rz": row_layout(Z8[rows]),
            "cq": col_layout(Q8[rows]), "cx": col_layout(X8[rows]),
            "s1p": vec_layout(dy1p[rows].astype(ml_dtypes.float8_e4m3)),
            "s1z": vec_layout(dy1z[rows].astype(ml_dtypes.float8_e4m3)),
            "u1pre": u1pre[rows].reshape(1, RPC).astype(np.float32),
            "rsx": rsx[rows].reshape(1, RPC).astype(np.float32),
            "csp": csp_t, "csz": csz_t,
        })
    host = {"u1pre": u1pre, "rsx": rsx, "csp": csp, "csz": csz}
    return in_maps, host


def _make_in_maps(S, Z, X):
    in_maps, host = _host_stats(S, Z, X)
    _CACHED["host"] = host
    return in_maps


def _finale(res):
    """Assemble the scalar objective from device outputs (float64)."""
    host = _CACHED["host"]
    u1pre = host["u1pre"].astype(np.float64)
    rsx = host["rsx"].astype(np.float64)
    csp = host["csp"].astype(np.float64)
    csz = host["csz"].astype(np.float64)

    u2F = np.concatenate([np.asarray(res[i]["u2f"], np.float64).ravel()
                          for i in range(N_CORES)])
    w2F = np.concatenate([np.asarray(res[i]["w2f"], np.float64).ravel()
                          for i in range(N_CORES)])
    h2F = np.asarray(res[0]["h2f"], np.float64).ravel()
    vp = np.sum([np.asarray(res[i]["vp"], np.float64).ravel()
                 for i in range(N_CORES)], axis=0)

    u2 = u2F / (F * 2 ** 8)
    w2 = w2F / (F * 2 ** 8)
    h2 = h2F / (F * 2 ** 11)
    v2 = (C2Z * csz + vp / F) / 2 ** 11

    rs_sz = u1pre                       # = rsS + b*rsZ
    lR = np.log(R)
    term1 = R * (u2.sum() * h2.sum() + ALPHA * w2.sum() * h2.sum()
                 + BETA * u2.sum() * v2.sum())
    O = (term1
         - (csp.sum() + BETA * csz.sum()) * lR
         - (np.log(u2) * rs_sz).sum()
         - ALPHA * (np.log(w2) * rsx).sum()
         - (np.log(h2) * csp).sum()
         - BETA * (np.log(v2) * csz).sum())
    return np.float32(O)


def _numpy_fallback(S, Z, X, U, H, W, V):
    """Faithful CPU implementation (only used if factors are not all-ones)."""
    S, Z, X, U, H, W, V = [np.asarray(a, np.float32) for a in (S, Z, X, U, H, W, V)]

    def obj(Sp, Xp, Zp):
        return ((Sp - S * np.log(Sp)).sum()
                + ALPHA * (Xp - X * np.log(Xp)).sum()
                + BETA * (Zp - Z * np.log(Zp)).sum())

    Sp = U @ H; Xp = W @ H; Zp = U @ V
    Sd = S / Sp; Xd = X / Xp; Zd = Z / Zp
    O = obj(Sp, Xp, Zp)
    for _ in range(2):
        dHV = H + BETA * V
        U = U * (Sd @ (H / dHV).T + Zd @ ((BETA * V) / dHV).T)
        Sp = U @ H; Zp = U @ V; Sd = S / Sp; Zd = Z / Zp
        dUW = U + ALPHA * W
        H = H * ((U / dUW).T @ Sd + ((ALPHA * W) / dUW).T @ Xd)
        Sp = U @ H; Xp = W @ H; Sd = S / Sp; Xd = X / Xp
        W = W * Xd.sum(axis=1, keepdims=True)
        Xp = W @ H; Xd = X / Xp
        V = V * Zd.sum(axis=0, keepdims=True)
        Zp = U @ V; Zd = Z / Zp
        O = obj(Sp, Xp, Zp)
    return np.float32(O)


def kernel(S, Z, X, U, H, W, V):
    if not (np.all(np.asarray(U) == 1) and np.all(np.asarray(H) == 1)
            and np.all(np.asarray(W) == 1) and np.all(np.asarray(V) == 1)):
        return _numpy_fallback(S, Z, X, U, H, W, V)

    import time
    from concourse.bass_utils import run_bass_kernel_spmd

    if "nc" not in _CACHED:
        _CACHED["nc"] = _build()
    nc = _CACHED["nc"]

    in_maps = _make_in_maps(S, Z, X)
    last = None
    for attempt in range(3):
        try:
            res = run_bass_kernel_spmd(nc, in_maps, core_ids=list(range(N_CORES)))
            return _finale(res.results)
        except Exception as e:  # transient NRT/device errors: reset and retry
            last = e
            try:
                import jax
                jax.clear_caches()
                jax.clear_backends()
            except Exception:
                pass
            time.sleep(3.0)
    raise last


if __name__ == "__main__":
    import reference
    inputs = reference.setup_inputs()
    inputs = {k: np.asarray(v) for k, v in inputs.items()}
    print("kernel:", kernel(**inputs))



# revision 10
# speedup vs baseline: 1.4398x; 1.3137x over previous
"""Trainium2 8-core kernel for nn_ACCSLP_59485297050024 (column-sharded, 1 AllReduce).

The reference is a multiplicative-update NMF-style solver on N=4096 nodes with
rank R=128 and N_ITERS=2, returning a scalar objective O. With all-ones factor
inits the whole computation collapses exactly to rank-1 vector recurrences
(see kernel_v1 docstring for the derivation):

    stage1: h1 = P^T dy1p, v1 = Z^T dy1z          (dy* from host input stats)
    stage2: u2 = Q g1, w1 = X g2, g* = f(h1,v1)   (contraction over columns)
    stage3: h2 = P^T g3, v2 = Z^T g4, g* = f(u2,w1)
    stage4: w2 = X g5, g5 = f(h2)
    O = closed form in (u2, w2, h2, v2, input stats)   [host, float64]

P = S + a*X and Q = S + b*Z (S never appears alone). All matrices stream in
FP8 E4M3; reciprocal stationaries are stored as centered deltas y = c + dy/16
with compile-time centers (validated: objective rel err ~6e-4 vs f32 ref).

Sharding: COLUMN-shard all four matrices (each core owns 512 columns, all
4096 rows). Then stages 1 and 3 are fully LOCAL (contraction over rows is
within-core), and only stage 2 needs a cross-core reduction: one 32KB
AllReduce of the (u2, w1) partials. Stage-4 partials are summed on host
(8 x 16KB), like v2 was in the row-sharded version. This removes one of the
two AllReduces and minimizes the work that serializes after the collective
(the per-execution collective barrier dominates the pre-AR phase anyway).

Per core SBUF residency: P,Z col-shard row-major [128,32,512] (stages 1,3);
Q,X col-shard col-major [128,4,4096] (stages 2,4). Every contraction is a
TensorE DoubleRow fp8 matmul with a tiny stationary vector; paired streams
share one PSUM bank by placing their stationary values in different columns
(output partitions).
"""

import numpy as np
import ml_dtypes

N = 4096
R = 128
ALPHA = 0.5
BETA = 0.5
N_CORES = 8
CPC = N // N_CORES          # columns per core = 512
NG = N // 128               # row groups = 32 (16 DoubleRow pairs)
NJ8 = N // 512              # 512-wide free chunks = 8

F = 16.0                    # delta-stationary scale
# centered-delta constants (binary-exact in f32); y_scaled = c + dy/F
C1P = 0.96875               # y_s = 2^11/(R(u1+a))
C1Z = 1.0                   # y_s = 2^11/(R u1)
CQ = 1.015625               # y_s = 2^8/(R(h1+b v1))
CX = 1.359375               # y_s = 2^8/(R h1)
C2P = 0.921875              # y_s = 2^11/(R(u2+a w1))
C2Z = 1.328125              # y_s = 2^11/(R u2)
C3 = 1.421875               # y_s = 2^8/(R h2)

_CACHED = {}


def _build():
    import concourse.mybir as mybir
    import concourse.tile as tile
    from concourse import bacc
    from concourse.masks import make_identity

    f8 = mybir.dt.float8e4
    f32 = mybir.dt.float32
    DR = mybir.MatmulPerfMode.DoubleRow

    nc = bacc.Bacc("TRN2", target_bir_lowering=False, debug=False,
                   num_devices=N_CORES, dynamic_dma_scratch_size=8192)

    # per-core external inputs (host supplies per-partition-contiguous layouts)
    rp_e = nc.declare_dram_parameter("rp", [128, NG, CPC], f8, isOutput=False)
    rz_e = nc.declare_dram_parameter("rz", [128, NG, CPC], f8, isOutput=False)
    cq_e = nc.declare_dram_parameter("cq", [128, 4, N], f8, isOutput=False)
    cx_e = nc.declare_dram_parameter("cx", [128, 4, N], f8, isOutput=False)
    s1p_e = nc.declare_dram_parameter("s1p", [128, 16, 2, 32], f8, isOutput=False)
    s1z_e = nc.declare_dram_parameter("s1z", [128, 16, 2, 32], f8, isOutput=False)
    cs2_e = nc.declare_dram_parameter("cs2", [2, CPC], f32, isOutput=False)
    cs3_e = nc.declare_dram_parameter("cs3", [2, CPC], f32, isOutput=False)
    uwpre_e = nc.declare_dram_parameter("uwpre", [16, 512], f32, isOutput=False)
    # per-core external outputs
    out_u2 = nc.declare_dram_parameter("u2f", [8, 512], f32, isOutput=True)
    out_h2v2 = nc.declare_dram_parameter("h2v2", [2, CPC], f32, isOutput=True)
    out_w2p = nc.declare_dram_parameter("w2p", [8, 512], f32, isOutput=True)

    ar_out = nc.dram_tensor("ar_out", [16, 512], f32, addr_space="Shared")
    groups = [list(range(N_CORES))]

    with tile.TileContext(nc) as tc:
        with (
            tc.tile_pool(name="big", bufs=1) as big,
            tc.tile_pool(name="small", bufs=1) as small,
            tc.tile_pool(name="stg", bufs=2) as stg,
            tc.tile_pool(name="pacc", bufs=1, space="PSUM") as pacc,
            tc.tile_pool(name="pstream", bufs=2, space="PSUM") as pstream,
            tc.tile_pool(name="ptrans", bufs=1, space="PSUM") as ptrans,
            tc.tile_pool(name="dram", bufs=1, space="DRAM") as dram,
        ):
            # ---------- small inputs ----------
            s1p = small.tile([128, 16, 2, 32], f8, tag="s1p")
            s1z = small.tile([128, 16, 2, 32], f8, tag="s1z")
            cs2 = small.tile([2, CPC], f32, tag="cs2")
            cs3 = small.tile([2, CPC], f32, tag="cs3")
            uwpre = small.tile([16, 512], f32, tag="uwpre")
            for t, e in ((s1p, s1p_e), (s1z, s1z_e), (cs2, cs2_e),
                         (cs3, cs3_e), (uwpre, uwpre_e)):
                nc.sync.dma_start(t[:], e[:])

            ident = small.tile([128, 128], f32, tag="ident")
            make_identity(nc, ident[:])

            # device-filled fp8 stationaries (values land in one column per
            # variant; zero-fill once, off critical path)
            statQ = small.tile([128, 2, NJ8, 2, 32], f8, tag="statQ")
            statX = small.tile([128, 2, NJ8, 2, 32], f8, tag="statX")
            statP2 = small.tile([128, 16, 2, 32], f8, tag="statP2")
            statZ2 = small.tile([128, 16, 2, 32], f8, tag="statZ2")
            stat3 = small.tile([128, 2, NJ8, 2, 32], f8, tag="stat3")
            for t in (statQ, statX, statP2, statZ2, stat3):
                nc.gpsimd.memset(t[:], 0.0)

            # ---------- resident loads (pieces, ordered for pipelining) ----
            QP = 4
            tRp = [big.tile([128, 8, CPC], f8, name=f"tRp{q}", tag=f"tRp{q}") for q in range(QP)]
            tRz = [big.tile([128, 8, CPC], f8, name=f"tRz{q}", tag=f"tRz{q}") for q in range(QP)]
            tCq = [big.tile([128, 4, 1024], f8, name=f"tCq{q}", tag=f"tCq{q}") for q in range(QP)]
            tCx = [big.tile([128, 4, 1024], f8, name=f"tCx{q}", tag=f"tCx{q}") for q in range(QP)]
            for q in range(QP):
                nc.sync.dma_start(tRp[q][:], rp_e[:, 8 * q:8 * q + 8, :])
            for q in range(QP):
                nc.sync.dma_start(tRz[q][:], rz_e[:, 8 * q:8 * q + 8, :])
            for q in range(QP):
                nc.sync.dma_start(tCq[q][:], cq_e[:, :, 1024 * q:1024 * q + 1024])
            for q in range(QP):
                nc.sync.dma_start(tCx[q][:], cx_e[:, :, 1024 * q:1024 * q + 1024])

            def movR(pieces, i):
                """row-major moving pair i (groups 2i, 2i+1): [128, 2, 512]"""
                return pieces[i // 4][:, (i % 4) * 2:(i % 4) * 2 + 2, :]

            def movC(pieces, c8, i):
                """col-major moving, l-chunk c8, j-pair i: [128, 2, 512]"""
                w = (c8 % 2) * 512
                return pieces[c8 // 2][:, 2 * i:2 * i + 2, w:w + 512]

            # ---------- stage 1: h1, v1 (local col slices) ----------
            psS1 = pacc.tile([32, CPC], f32, tag="psS1")
            for i in range(16):
                nc.tensor.matmul(psS1[:], s1p[:, i, :, :], movR(tRp, i),
                                 start=(i == 0), stop=False, perf_mode=DR)
            for i in range(16):
                nc.tensor.matmul(psS1[:], s1z[:, i, :, :], movR(tRz, i),
                                 start=False, stop=(i == 15), perf_mode=DR)
            # rows: 0 = dP (h1 part), 1 = dZ (v1 part)
            s1out = stg.tile([2, CPC], f32, tag="s1out")
            nc.scalar.copy(s1out[:], psS1[0:2, :])
            # h1F = F*C1P*csp + dP ; v1F = F*C1Z*csz + dZ   (cs2 host-prescaled)
            h1v1 = stg.tile([2, CPC], f32, tag="h1v1")
            nc.vector.scalar_tensor_tensor(h1v1[:], cs2[:], F, s1out[:],
                                           mybir.AluOpType.mult, mybir.AluOpType.add)
            # transpose first (PE crosses partitions), then combine in free dim:
            # ps_t2 cols k+4r: r=0 -> h1F by group g=k, r=1 -> v1F at cols 4..7
            ps_t2 = ptrans.tile([128, 8], f32, tag="pt")
            for k in range(4):
                nc.tensor.transpose(ps_t2[:, k:8:4], h1v1[:, 128 * k:128 * (k + 1)],
                                    ident[0:2, 0:2])
            # combo cols 0:4 = tQ2 = h1F + b*v1F ; cols 4:8 = h1F
            tp2 = stg.tile([128, 8], f32, tag="tp2")
            nc.vector.tensor_copy(tp2[:], ps_t2[:])
            combo2 = stg.tile([128, 8], f32, tag="combo2")
            nc.vector.scalar_tensor_tensor(combo2[:, 0:4], tp2[:, 4:8], BETA,
                                           tp2[:, 0:4],
                                           mybir.AluOpType.mult, mybir.AluOpType.add)
            nc.vector.tensor_copy(combo2[:, 4:8], tp2[:, 0:4])
            pre_a = small.tile([128, 8], f32, tag="pre_a")
            nc.vector.reciprocal(pre_a[:], combo2[:])
            # statQ: y-values in column c8 (-> psum row c8); statX at c8+8
            for c8 in range(NJ8):
                nc.vector.tensor_scalar(
                    statQ[:, :, c8, :, c8],
                    pre_a[:, 0:4].rearrange("p (c j) -> p c j", j=2),
                    float(2 ** 20), -F * CQ,
                    mybir.AluOpType.mult, mybir.AluOpType.add)
                nc.vector.tensor_scalar(
                    statX[:, :, c8, :, c8 + 8],
                    pre_a[:, 4:8].rearrange("p (c j) -> p c j", j=2),
                    float(2 ** 20), -F * CX,
                    mybir.AluOpType.mult, mybir.AluOpType.add)

            # ---------- stage 2: u2, w1 partials + AllReduce ----------
            ar_in = dram.tile([16, 512], f32, tag="ar_in")
            psQX = pstream.tile([32, 512], f32, tag="psQX")
            for c8 in range(NJ8):
                for i in range(2):
                    nc.tensor.matmul(psQX[:], statQ[:, i, c8, :, :], movC(tCq, c8, i),
                                     start=(c8 == 0 and i == 0), stop=False,
                                     perf_mode=DR)
            for c8 in range(NJ8):
                for i in range(2):
                    nc.tensor.matmul(psQX[:], statX[:, i, c8, :, :], movC(tCx, c8, i),
                                     start=False, stop=(c8 == NJ8 - 1 and i == 1),
                                     perf_mode=DR)
            stQX = stg.tile([16, 512], f32, tag="stQX")
            nc.scalar.copy(stQX[:], psQX[0:16, :])
            nc.gpsimd.dma_start(ar_in[:], stQX[:])
            nc.gpsimd.collective_compute(
                "AllReduce", mybir.AluOpType.add, replica_groups=groups,
                ins=[ar_in.opt()], outs=[ar_out[:].opt()])

            # ---------- stage 3 stationaries from AR ----------
            ar_rd = stg.tile([16, 512], f32, tag="ar_rd")
            nc.gpsimd.dma_start(ar_rd[:], ar_out[:])
            # uw rows 0-7: u2F = F*CQ*u1pre + arQ ; rows 8-15: w1F = F*CX*rsx + arX
            uw = small.tile([16, 512], f32, tag="uw")
            nc.vector.scalar_tensor_tensor(uw[:], uwpre[:], F, ar_rd[:],
                                           mybir.AluOpType.mult, mybir.AluOpType.add)
            nc.sync.dma_start(out_u2[:], uw[0:8, :])
            # transpose first: ps_t3 cols k+4r: r 0-7 -> u2F by group g=4r+k
            # (cols 0:32), r 8-15 -> w1F at cols 32+g
            ps_t3 = ptrans.tile([128, 64], f32, tag="pt")
            for k in range(4):
                nc.tensor.transpose(ps_t3[:, k:64:4], uw[:, 128 * k:128 * (k + 1)],
                                    ident[0:16, 0:16])
            # combo cols 0:32 = t2F = u2F + a*w1F ; cols 32:64 = u2F
            tp3 = stg.tile([128, 64], f32, tag="tp3")
            nc.vector.tensor_copy(tp3[:], ps_t3[:])
            combo3 = stg.tile([128, 64], f32, tag="combo3")
            nc.vector.scalar_tensor_tensor(combo3[:, 0:32], tp3[:, 32:64], ALPHA,
                                           tp3[:, 0:32],
                                           mybir.AluOpType.mult, mybir.AluOpType.add)
            nc.vector.tensor_copy(combo3[:, 32:64], tp3[:, 0:32])
            pre_b = small.tile([128, 64], f32, tag="pre_b")
            nc.vector.reciprocal(pre_b[:], combo3[:])
            nc.vector.tensor_scalar(
                statP2[:, :, :, 0],
                pre_b[:, 0:32].rearrange("p (c j) -> p c j", j=2),
                float(2 ** 20), -F * C2P,
                mybir.AluOpType.mult, mybir.AluOpType.add)
            nc.vector.tensor_scalar(
                statZ2[:, :, :, 1],
                pre_b[:, 32:64].rearrange("p (c j) -> p c j", j=2),
                float(2 ** 20), -F * C2Z,
                mybir.AluOpType.mult, mybir.AluOpType.add)

            # ---------- stage 3: h2, v2 (local col slices) ----------
            psS3 = pstream.tile([32, CPC], f32, tag="psS3")
            for i in range(16):
                nc.tensor.matmul(psS3[:], statP2[:, i, :, :], movR(tRp, i),
                                 start=(i == 0), stop=False, perf_mode=DR)
            for i in range(16):
                nc.tensor.matmul(psS3[:], statZ2[:, i, :, :], movR(tRz, i),
                                 start=False, stop=(i == 15), perf_mode=DR)
            # h2F = F*C2P*csp + dP2 ; v2F = F*C2Z*csz + dZ2
            h2v2 = stg.tile([2, CPC], f32, tag="h2v2")
            nc.vector.scalar_tensor_tensor(h2v2[:], cs3[:], F, psS3[0:2, :],
                                           mybir.AluOpType.mult, mybir.AluOpType.add)
            nc.sync.dma_start(out_h2v2[:], h2v2[:])

            # ---------- stage 4: w2 partial ----------
            ps_t4 = ptrans.tile([128, 8], f32, tag="pt")
            for k in range(4):
                nc.tensor.transpose(ps_t4[:, k:8:4], h2v2[:, 128 * k:128 * (k + 1)],
                                    ident[0:2, 0:2])
            pre_3 = small.tile([128, 4], f32, tag="pre_3")
            nc.vector.reciprocal(pre_3[:], ps_t4[:, 0:4])
            for c8 in range(NJ8):
                nc.vector.tensor_scalar(
                    stat3[:, :, c8, :, c8],
                    pre_3[:, 0:4].rearrange("p (c j) -> p c j", j=2),
                    float(2 ** 20), -F * C3,
                    mybir.AluOpType.mult, mybir.AluOpType.add)
            psX4 = pstream.tile([32, 512], f32, tag="psQX")
            for c8 in range(NJ8):
                for i in range(2):
                    nc.tensor.matmul(psX4[:], stat3[:, i, c8, :, :], movC(tCx, c8, i),
                                     start=(c8 == 0 and i == 0),
                                     stop=(c8 == NJ8 - 1 and i == 1), perf_mode=DR)
            stW = stg.tile([8, 512], f32, tag="stW")
            nc.scalar.copy(stW[:], psX4[0:8, :])
            nc.sync.dma_start(out_w2p[:], stW[:])

    nc.compile()
    return nc


def _host_stats(S, Z, X):
    """fp8 casts + input statistics; returns per-core in_maps and host data."""
    S = np.asarray(S, np.float32)
    Z = np.asarray(Z, np.float32)
    X = np.asarray(X, np.float32)
    P8 = (S + ALPHA * X).astype(ml_dtypes.float8_e4m3)
    Q8 = (S + BETA * Z).astype(ml_dtypes.float8_e4m3)
    X8 = X.astype(ml_dtypes.float8_e4m3)
    Z8 = Z.astype(ml_dtypes.float8_e4m3)

    Pf = P8.astype(np.float32)
    Qf = Q8.astype(np.float32)
    Xf = X8.astype(np.float32)
    Zf = Z8.astype(np.float32)
    u1pre = Qf.sum(axis=1)                  # rowsum(Q)  (N,)
    rsx = Xf.sum(axis=1)                    # rowsum(X)
    csp = Pf.sum(axis=0)                    # colsum(P)  (N,)
    csz = Zf.sum(axis=0)

    qq = u1pre * (2.0 / 3.0)                # R*u1
    dy1p = ((2048.0 / (qq + 64.0)) - C1P) * F
    dy1z = ((2048.0 / qq) - C1Z) * F

    def stat_layout(v, col):
        # [4096] -> [128(p), 16(pair), 2(k), 32(col)], values at `col`
        g = v.reshape(NG, 128).T                 # [128, 32]; l = g*128 + p
        out = np.zeros((128, 16, 2, 32), v.dtype)
        out[:, :, 0, col] = g[:, 0::2]
        out[:, :, 1, col] = g[:, 1::2]
        return out

    s1p = stat_layout(dy1p.astype(ml_dtypes.float8_e4m3), 0)
    s1z = stat_layout(dy1z.astype(ml_dtypes.float8_e4m3), 1)

    def row_layout(colshard):
        # [4096, 512] -> [128(p), NG(g), 512(j)]; row l = g*128 + p
        return np.ascontiguousarray(
            colshard.reshape(NG, 128, CPC).transpose(1, 0, 2))

    def col_layout(colshard):
        # [4096, 512] -> [128(p), 4(a), 4096(l)]; col j_local = a*128 + p
        return np.ascontiguousarray(
            colshard.T.reshape(4, 128, N).transpose(1, 0, 2))

    uwpre = np.concatenate([CQ * u1pre, CX * rsx]).reshape(16, 512).astype(np.float32)

    in_maps = []
    for c in range(N_CORES):
        cols = slice(c * CPC, (c + 1) * CPC)
        cs2 = np.stack([C1P * csp[cols], C1Z * csz[cols]]).astype(np.float32)
        cs3 = np.stack([C2P * csp[cols], C2Z * csz[cols]]).astype(np.float32)
        in_maps.append({
            "rp": row_layout(P8[:, cols]), "rz": row_layout(Z8[:, cols]),
            "cq": col_layout(Q8[:, cols]), "cx": col_layout(X8[:, cols]),
            "s1p": s1p, "s1z": s1z,
            "cs2": np.ascontiguousarray(cs2),
            "cs3": np.ascontiguousarray(cs3),
            "uwpre": uwpre,
        })
    host = {"u1pre": u1pre, "rsx": rsx, "csp": csp, "csz": csz}
    return in_maps, host


def _make_in_maps(S, Z, X):
    in_maps, host = _host_stats(S, Z, X)
    _CACHED["host"] = host
    return in_maps


def _finale(res):
    """Assemble the scalar objective from device outputs (float64)."""
    host = _CACHED["host"]
    u1pre = host["u1pre"].astype(np.float64)
    rsx = host["rsx"].astype(np.float64)
    csp = host["csp"].astype(np.float64)
    csz = host["csz"].astype(np.float64)

    u2F = np.asarray(res[0]["u2f"], np.float64).ravel()
    h2F = np.concatenate([np.asarray(res[c]["h2v2"], np.float64)[0]
                          for c in range(N_CORES)])
    v2F = np.concatenate([np.asarray(res[c]["h2v2"], np.float64)[1]
                          for c in range(N_CORES)])
    w2F = C3 * F * rsx + np.sum(
        [np.asarray(res[c]["w2p"], np.float64).ravel() for c in range(N_CORES)],
        axis=0)

    u2 = u2F / (F * 2 ** 8)
    w2 = w2F / (F * 2 ** 8)
    h2 = h2F / (F * 2 ** 11)
    v2 = v2F / (F * 2 ** 11)

    rs_sz = u1pre                       # = rsS + b*rsZ
    lR = np.log(R)
    term1 = R * (u2.sum() * h2.sum() + ALPHA * w2.sum() * h2.sum()
                 + BETA * u2.sum() * v2.sum())
    O = (term1
         - (csp.sum() + BETA * csz.sum()) * lR
         - (np.log(u2) * rs_sz).sum()
         - ALPHA * (np.log(w2) * rsx).sum()
         - (np.log(h2) * csp).sum()
         - BETA * (np.log(v2) * csz).sum())
    return np.float32(O)


def _numpy_fallback(S, Z, X, U, H, W, V):
    """Faithful CPU implementation (only used if factors are not all-ones)."""
    S, Z, X, U, H, W, V = [np.asarray(a, np.float32) for a in (S, Z, X, U, H, W, V)]

    def obj(Sp, Xp, Zp):
        return ((Sp - S * np.log(Sp)).sum()
                + ALPHA * (Xp - X * np.log(Xp)).sum()
                + BETA * (Zp - Z * np.log(Zp)).sum())

    Sp = U @ H; Xp = W @ H; Zp = U @ V
    Sd = S / Sp; Xd = X / Xp; Zd = Z / Zp
    O = obj(Sp, Xp, Zp)
    for _ in range(2):
        dHV = H + BETA * V
        U = U * (Sd @ (H / dHV).T + Zd @ ((BETA * V) / dHV).T)
        Sp = U @ H; Zp = U @ V; Sd = S / Sp; Zd = Z / Zp
        dUW = U + ALPHA * W
        H = H * ((U / dUW).T @ Sd + ((ALPHA * W) / dUW).T @ Xd)
        Sp = U @ H; Xp = W @ H; Sd = S / Sp; Xd = X / Xp
        W = W * Xd.sum(axis=1, keepdims=True)
        Xp = W @ H; Xd = X / Xp
        V = V * Zd.sum(axis=0, keepdims=True)
        Zp = U @ V; Zd = Z / Zp
        O = obj(Sp, Xp, Zp)
    return np.float32(O)


def kernel(S, Z, X, U, H, W, V):
    if not (np.all(np.asarray(U) == 1) and np.all(np.asarray(H) == 1)
            and np.all(np.asarray(W) == 1) and np.all(np.asarray(V) == 1)):
        return _numpy_fallback(S, Z, X, U, H, W, V)

    import time
    from concourse.bass_utils import run_bass_kernel_spmd

    if "nc" not in _CACHED:
        _CACHED["nc"] = _build()
    nc = _CACHED["nc"]

    in_maps = _make_in_maps(S, Z, X)
    last = None
    for attempt in range(3):
        try:
            res = run_bass_kernel_spmd(nc, in_maps, core_ids=list(range(N_CORES)))
            return _finale(res.results)
        except Exception as e:  # transient NRT/device errors: reset and retry
            last = e
            try:
                import jax
                jax.clear_caches()
                jax.clear_backends()
            except Exception:
                pass
            time.sleep(3.0)
    raise last


if __name__ == "__main__":
    import reference
    inputs = reference.setup_inputs()
    inputs = {k: np.asarray(v) for k, v in inputs.items()}
    print("kernel:", kernel(**inputs))


# revision 12
# speedup vs baseline: 1.4617x; 1.0152x over previous
"""Trainium2 8-core kernel for nn_ACCSLP_59485297050024 (column-sharded, 1 AllReduce).

The reference is a multiplicative-update NMF-style solver on N=4096 nodes with
rank R=128 and N_ITERS=2, returning a scalar objective O. With all-ones factor
inits the whole computation collapses exactly to rank-1 vector recurrences:

    stage1: h1 = P^T dy1p, v1 = Z^T dy1z          (dy* from host input stats)
    stage2: u2 = Q g1, w1 = X g2, g* = f(h1,v1)   (contraction over columns)
    stage3: h2 = P^T g3, v2 = Z^T g4, g* = f(u2,w1)
    stage4: w2 = X g5, g5 = f(h2)
    O = closed form in (u2, w2, h2, v2, input stats)   [host, float64]

P = S + a*X and Q = S + b*Z (S never appears alone). All matrices stream in
FP8 E4M3; reciprocal stationaries are stored as centered deltas y = c + dy/16
with compile-time centers (validated: objective rel err ~6e-4 vs f32 ref).

Sharding: COLUMN-shard all four matrices (each core owns 512 columns, all
4096 rows). Stages 1 and 3 are then fully LOCAL and only stage 2 needs a
cross-core reduction: one 32KB AllReduce of the (u2, w1) partials. Stage-4
partials are summed on host (8 x 16KB). This leaves a single collective, and
everything after it is arranged to start matmuls as fast as possible:

  - the AR payload is PRE-transposed (PE transposes run in the pre-AR slack),
    so the readback is already partition-major and stage-3 stationaries need
    only vector ops (no PE work) after the AR;
  - the readback DMA runs on the sync queue (the gpsimd queue sits behind a
    ~2us post-collective drain);
  - statP2 is filled before statZ2 so the P stream issues immediately; the
    stage-4 stationary prep overlaps the Z stream via split PSUM banks.
"""

import numpy as np
import ml_dtypes

N = 4096
R = 128
ALPHA = 0.5
BETA = 0.5
N_CORES = 8
CPC = N // N_CORES          # columns per core = 512
NG = N // 128               # row groups = 32 (16 DoubleRow pairs)
NJ8 = N // 512              # 512-wide free chunks = 8

F = 16.0                    # delta-stationary scale
# centered-delta constants (binary-exact in f32); y_scaled = c + dy/F
C1P = 0.96875               # y_s = 2^11/(R(u1+a))
C1Z = 1.0                   # y_s = 2^11/(R u1)
CQ = 1.015625               # y_s = 2^8/(R(h1+b v1))
CX = 1.359375               # y_s = 2^8/(R h1)
C2P = 0.921875              # y_s = 2^11/(R(u2+a w1))
C2Z = 1.328125              # y_s = 2^11/(R u2)
C3 = 1.421875               # y_s = 2^8/(R h2)

_CACHED = {}


def _build():
    import concourse.mybir as mybir
    import concourse.tile as tile
    from concourse import bacc
    from concourse.masks import make_identity

    f8 = mybir.dt.float8e4
    f32 = mybir.dt.float32
    DR = mybir.MatmulPerfMode.DoubleRow
    MUL = mybir.AluOpType.mult
    ADD = mybir.AluOpType.add

    nc = bacc.Bacc("TRN2", target_bir_lowering=False, debug=False,
                   num_devices=N_CORES, dynamic_dma_scratch_size=8192)

    # per-core external inputs (host supplies per-partition-contiguous layouts)
    rp_e = nc.declare_dram_parameter("rp", [128, NG, CPC], f8, isOutput=False)
    rz_e = nc.declare_dram_parameter("rz", [128, NG, CPC], f8, isOutput=False)
    cq_e = nc.declare_dram_parameter("cq", [128, 4, N], f8, isOutput=False)
    cx_e = nc.declare_dram_parameter("cx", [128, 4, N], f8, isOutput=False)
    s1p_e = nc.declare_dram_parameter("s1p", [128, 16, 2, 32], f8, isOutput=False)
    s1z_e = nc.declare_dram_parameter("s1z", [128, 16, 2, 32], f8, isOutput=False)
    cs2_e = nc.declare_dram_parameter("cs2", [2, CPC], f32, isOutput=False)
    cs3p_e = nc.declare_dram_parameter("cs3p", [1, CPC], f32, isOutput=False)
    cs3z_e = nc.declare_dram_parameter("cs3z", [1, CPC], f32, isOutput=False)
    uwpre_e = nc.declare_dram_parameter("uwpre", [128, 64], f32, isOutput=False)
    # per-core external outputs
    out_u2 = nc.declare_dram_parameter("u2f", [128, 32], f32, isOutput=True)
    out_h2 = nc.declare_dram_parameter("h2f", [1, CPC], f32, isOutput=True)
    out_v2 = nc.declare_dram_parameter("v2f", [1, CPC], f32, isOutput=True)
    out_w2p = nc.declare_dram_parameter("w2p", [8, 512], f32, isOutput=True)

    ar_out = nc.dram_tensor("ar_out", [128, 64], f32, addr_space="Shared")
    groups = [list(range(N_CORES))]

    with tile.TileContext(nc) as tc:
        with (
            tc.tile_pool(name="res", bufs=1) as res,
            tc.tile_pool(name="pacc", bufs=1, space="PSUM") as pacc,
            tc.tile_pool(name="ptrans", bufs=1, space="PSUM") as ptrans,
            tc.tile_pool(name="dram", bufs=1, space="DRAM") as dram,
        ):
            # ---------- small inputs ----------
            s1p = res.tile([128, 16, 2, 32], f8, tag="s1p")
            s1z = res.tile([128, 16, 2, 32], f8, tag="s1z")
            cs2 = res.tile([2, CPC], f32, tag="cs2")
            cs3p = res.tile([1, CPC], f32, tag="cs3p")
            cs3z = res.tile([1, CPC], f32, tag="cs3z")
            uwpre = res.tile([128, 64], f32, tag="uwpre")
            for t, e in ((s1p, s1p_e), (s1z, s1z_e), (cs2, cs2_e),
                         (cs3p, cs3p_e), (cs3z, cs3z_e), (uwpre, uwpre_e)):
                nc.sync.dma_start(t[:], e[:])

            ident = res.tile([128, 128], f32, tag="ident")
            make_identity(nc, ident[:])

            # device-filled fp8 stationaries (values land in one column per
            # variant; zero-fill once, off critical path)
            statQ = res.tile([128, 2, NJ8, 2, 32], f8, tag="statQ")
            statX = res.tile([128, 2, NJ8, 2, 32], f8, tag="statX")
            statP2 = res.tile([128, 16, 2, 32], f8, tag="statP2")
            statZ2 = res.tile([128, 16, 2, 32], f8, tag="statZ2")
            stat3 = res.tile([128, 2, NJ8, 2, 32], f8, tag="stat3")
            for t in (statQ, statX, statP2, statZ2, stat3):
                nc.gpsimd.memset(t[:], 0.0)

            # ---------- resident loads (pieces, ordered for pipelining) ----
            tRp = [res.tile([128, 8, CPC], f8, name=f"tRp{q}", tag=f"tRp{q}") for q in range(4)]
            tRz = [res.tile([128, 16, CPC], f8, name=f"tRz{q}", tag=f"tRz{q}") for q in range(2)]
            tCq = [res.tile([128, 4, 2048], f8, name=f"tCq{q}", tag=f"tCq{q}") for q in range(2)]
            tCx = [res.tile([128, 4, 2048], f8, name=f"tCx{q}", tag=f"tCx{q}") for q in range(2)]
            for q in range(4):
                nc.sync.dma_start(tRp[q][:], rp_e[:, 8 * q:8 * q + 8, :])
            for q in range(2):
                nc.sync.dma_start(tRz[q][:], rz_e[:, 16 * q:16 * q + 16, :])
            for q in range(2):
                nc.sync.dma_start(tCq[q][:], cq_e[:, :, 2048 * q:2048 * q + 2048])
            for q in range(2):
                nc.sync.dma_start(tCx[q][:], cx_e[:, :, 2048 * q:2048 * q + 2048])

            def movRp(i):
                return tRp[i // 4][:, (i % 4) * 2:(i % 4) * 2 + 2, :]

            def movRz(i):
                return tRz[i // 8][:, (i % 8) * 2:(i % 8) * 2 + 2, :]

            def movC(pieces, c8, i):
                w = (c8 % 4) * 512
                return pieces[c8 // 4][:, 2 * i:2 * i + 2, w:w + 512]

            # ---------- stage 1: h1, v1 (local col slices) ----------
            psS1 = pacc.tile([32, CPC], f32, tag="psS1")
            for i in range(16):
                nc.tensor.matmul(psS1[:], s1p[:, i, :, :], movRp(i),
                                 start=(i == 0), stop=False, perf_mode=DR)
            for i in range(16):
                nc.tensor.matmul(psS1[:], s1z[:, i, :, :], movRz(i),
                                 start=False, stop=(i == 15), perf_mode=DR)
            # rows: 0 = dP (h1 part), 1 = dZ (v1 part)
            s1out = res.tile([2, CPC], f32, tag="s1out")
            nc.scalar.copy(s1out[:], psS1[0:2, :])
            # h1F = F*C1P*csp + dP ; v1F = F*C1Z*csz + dZ   (cs2 host-prescaled)
            h1v1 = res.tile([2, CPC], f32, tag="h1v1")
            nc.vector.scalar_tensor_tensor(h1v1[:], cs2[:], F, s1out[:], MUL, ADD)
            # transpose (PE crosses partitions): ps_t cols k+4r, r=0 -> h1F by
            # group g=k (cols 0:4), r=1 -> v1F (cols 4:8)
            ps_t2 = ptrans.tile([128, 64], f32, tag="pt")
            for k in range(4):
                nc.tensor.transpose(ps_t2[:, k:8:4], h1v1[:, 128 * k:128 * (k + 1)],
                                    ident[0:2, 0:2])
            tp2 = res.tile([128, 8], f32, tag="tp2")
            nc.vector.tensor_copy(tp2[:], ps_t2[:, 0:8])
            pre_a = res.tile([128, 8], f32, tag="pre_a")
            # cols 0:4 = 1/(h1F + b*v1F), cols 4:8 = 1/h1F
            tq2t = res.tile([128, 4], f32, tag="tq2t")
            nc.vector.scalar_tensor_tensor(tq2t[:], tp2[:, 4:8], BETA, tp2[:, 0:4],
                                           MUL, ADD)
            nc.vector.reciprocal(pre_a[:, 0:4], tq2t[:])
            nc.vector.reciprocal(pre_a[:, 4:8], tp2[:, 0:4])
            # statQ: y-values in column c8 (-> psum row c8); statX at c8+8
            for c8 in range(NJ8):
                nc.vector.tensor_scalar(
                    statQ[:, :, c8, :, c8],
                    pre_a[:, 0:4].rearrange("p (c j) -> p c j", j=2),
                    float(2 ** 20), -F * CQ, MUL, ADD)
                nc.vector.tensor_scalar(
                    statX[:, :, c8, :, c8 + 8],
                    pre_a[:, 4:8].rearrange("p (c j) -> p c j", j=2),
                    float(2 ** 20), -F * CX, MUL, ADD)

            # ---------- stage 2: u2, w1 partials ----------
            psQX = pacc.tile([32, 512], f32, tag="psQX")
            for c8 in range(NJ8):
                for i in range(2):
                    nc.tensor.matmul(psQX[:], statQ[:, i, c8, :, :], movC(tCq, c8, i),
                                     start=(c8 == 0 and i == 0), stop=False,
                                     perf_mode=DR)
            for c8 in range(NJ8):
                for i in range(2):
                    nc.tensor.matmul(psQX[:], statX[:, i, c8, :, :], movC(tCx, c8, i),
                                     start=False, stop=(c8 == NJ8 - 1 and i == 1),
                                     perf_mode=DR)
            # pre-transpose the AR payload (pre-AR slack): rows 0-7 = u2
            # partial chunks, 8-15 = w1 -> [128, 64] partition-major
            stQX = res.tile([16, 512], f32, tag="stQX")
            nc.scalar.copy(stQX[:], psQX[0:16, :])
            ps_tq = ptrans.tile([128, 64], f32, tag="pt")
            for k in range(4):
                nc.tensor.transpose(ps_tq[:, k:64:4], stQX[:, 128 * k:128 * (k + 1)],
                                    ident[0:16, 0:16])
            arr = res.tile([128, 64], f32, tag="arr")
            nc.vector.tensor_copy(arr[:], ps_tq[:])
            ar_in = dram.tile([128, 64], f32, tag="ar_in")
            nc.gpsimd.dma_start(ar_in[:], arr[:])
            nc.gpsimd.collective_compute(
                "AllReduce", mybir.AluOpType.add, replica_groups=groups,
                ins=[ar_in.opt()], outs=[ar_out[:].opt()])

            # ---------- stage 3 stationaries from AR (vector-only) ----------
            ar_rd = res.tile([128, 64], f32, tag="ar_rd")
            nc.sync.dma_start(ar_rd[:], ar_out[:])
            # cols 0:32: u2F = F*CQ*u1pre + arQ ; 32:64: w1F = F*CX*rsx + arX
            uw = res.tile([128, 64], f32, tag="uw")
            nc.vector.scalar_tensor_tensor(uw[:], uwpre[:], F, ar_rd[:], MUL, ADD)
            t2t = res.tile([128, 32], f32, tag="t2t")
            nc.vector.scalar_tensor_tensor(t2t[:], uw[:, 32:64], ALPHA, uw[:, 0:32],
                                           MUL, ADD)
            pre_b = res.tile([128, 64], f32, tag="pre_b")
            nc.vector.reciprocal(pre_b[:, 0:32], t2t[:])
            nc.vector.tensor_scalar(
                statP2[:, :, :, 0],
                pre_b[:, 0:32].rearrange("p (c j) -> p c j", j=2),
                float(2 ** 20), -F * C2P, MUL, ADD)
            nc.sync.dma_start(out_u2[:], uw[:, 0:32])

            # ---------- stage 3: h2 (P stream starts asap) ----------
            psS3a = pacc.tile([32, CPC], f32, tag="psS3a")
            for i in range(16):
                nc.tensor.matmul(psS3a[:], statP2[:, i, :, :], movRp(i),
                                 start=(i == 0), stop=(i == 15), perf_mode=DR)
            # v2 stationaries fill during the P stream
            nc.vector.reciprocal(pre_b[:, 32:64], uw[:, 0:32])
            nc.vector.tensor_scalar(
                statZ2[:, :, :, 0],
                pre_b[:, 32:64].rearrange("p (c j) -> p c j", j=2),
                float(2 ** 20), -F * C2Z, MUL, ADD)
            # h2F = F*C2P*csp + dP2 ; stage-4 prep overlaps the Z stream below
            h2f = res.tile([1, CPC], f32, tag="h2f")
            nc.vector.scalar_tensor_tensor(h2f[:], cs3p[:], F, psS3a[0:1, :], MUL, ADD)
            nc.sync.dma_start(out_h2[:], h2f[:])

            # ---------- stage 3b: v2 (separate bank) ----------
            psS3b = pacc.tile([32, CPC], f32, tag="psS3b")
            for i in range(16):
                nc.tensor.matmul(psS3b[:], statZ2[:, i, :, :], movRz(i),
                                 start=(i == 0), stop=(i == 15), perf_mode=DR)

            # ---------- stage 4: w2 partial (prep overlaps the Z stream) ----
            ps_t4 = ptrans.tile([128, 64], f32, tag="pt")
            for k in range(4):
                nc.tensor.transpose(ps_t4[:, k:k + 1], h2f[:, 128 * k:128 * (k + 1)],
                                    ident[0:1, 0:1])
            pre_3 = res.tile([128, 4], f32, tag="pre_3")
            nc.vector.reciprocal(pre_3[:], ps_t4[:, 0:4])
            for c8 in range(NJ8):
                nc.vector.tensor_scalar(
                    stat3[:, :, c8, :, c8],
                    pre_3[:, 0:4].rearrange("p (c j) -> p c j", j=2),
                    float(2 ** 20), -F * C3, MUL, ADD)
            # v2 output (vector op after the stat3 fills so it can't stall them)
            v2f = res.tile([1, CPC], f32, tag="v2f")
            nc.vector.scalar_tensor_tensor(v2f[:], cs3z[:], F, psS3b[0:1, :], MUL, ADD)
            nc.sync.dma_start(out_v2[:], v2f[:])
            psX4 = pacc.tile([32, 512], f32, tag="psX4")
            for c8 in range(NJ8):
                for i in range(2):
                    nc.tensor.matmul(psX4[:], stat3[:, i, c8, :, :], movC(tCx, c8, i),
                                     start=(c8 == 0 and i == 0),
                                     stop=(c8 == NJ8 - 1 and i == 1), perf_mode=DR)
            stW = res.tile([8, 512], f32, tag="stW")
            nc.scalar.copy(stW[:], psX4[0:8, :])
            nc.sync.dma_start(out_w2p[:], stW[:])

    nc.compile()
    return nc


def _host_stats(S, Z, X):
    """fp8 casts + input statistics; returns per-core in_maps and host data."""
    S = np.asarray(S, np.float32)
    Z = np.asarray(Z, np.float32)
    X = np.asarray(X, np.float32)
    P8 = (S + ALPHA * X).astype(ml_dtypes.float8_e4m3)
    Q8 = (S + BETA * Z).astype(ml_dtypes.float8_e4m3)
    X8 = X.astype(ml_dtypes.float8_e4m3)
    Z8 = Z.astype(ml_dtypes.float8_e4m3)

    Pf = P8.astype(np.float32)
    Qf = Q8.astype(np.float32)
    Xf = X8.astype(np.float32)
    Zf = Z8.astype(np.float32)
    u1pre = Qf.sum(axis=1)                  # rowsum(Q)  (N,)
    rsx = Xf.sum(axis=1)                    # rowsum(X)
    csp = Pf.sum(axis=0)                    # colsum(P)  (N,)
    csz = Zf.sum(axis=0)

    qq = u1pre * (2.0 / 3.0)                # R*u1
    dy1p = ((2048.0 / (qq + 64.0)) - C1P) * F
    dy1z = ((2048.0 / qq) - C1Z) * F

    def stat_layout(v, col):
        # [4096] -> [128(p), 16(pair), 2(k), 32(col)], values at `col`
        g = v.reshape(NG, 128).T                 # [128, 32]; l = g*128 + p
        out = np.zeros((128, 16, 2, 32), v.dtype)
        out[:, :, 0, col] = g[:, 0::2]
        out[:, :, 1, col] = g[:, 1::2]
        return out

    s1p = stat_layout(dy1p.astype(ml_dtypes.float8_e4m3), 0)
    s1z = stat_layout(dy1z.astype(ml_dtypes.float8_e4m3), 1)

    def row_layout(colshard):
        # [4096, 512] -> [128(p), NG(g), 512(j)]; row l = g*128 + p
        return np.ascontiguousarray(
            colshard.reshape(NG, 128, CPC).transpose(1, 0, 2))

    def col_layout(colshard):
        # [4096, 512] -> [128(p), 4(a), 4096(l)]; col j_local = a*128 + p
        return np.ascontiguousarray(
            colshard.T.reshape(4, 128, N).transpose(1, 0, 2))

    # [128, 64]: cols 0:32 = CQ*u1pre, 32:64 = CX*rsx, both at (p,g)=l=g*128+p
    uwpre = np.concatenate(
        [CQ * u1pre.reshape(32, 128).T, CX * rsx.reshape(32, 128).T],
        axis=1).astype(np.float32)

    in_maps = []
    for c in range(N_CORES):
        cols = slice(c * CPC, (c + 1) * CPC)
        cs2 = np.stack([C1P * csp[cols], C1Z * csz[cols]]).astype(np.float32)
        in_maps.append({
            "rp": row_layout(P8[:, cols]), "rz": row_layout(Z8[:, cols]),
            "cq": col_layout(Q8[:, cols]), "cx": col_layout(X8[:, cols]),
            "s1p": s1p, "s1z": s1z,
            "cs2": np.ascontiguousarray(cs2),
            "cs3p": np.ascontiguousarray(C2P * csp[cols]).reshape(1, CPC).astype(np.float32),
            "cs3z": np.ascontiguousarray(C2Z * csz[cols]).reshape(1, CPC).astype(np.float32),
            "uwpre": np.ascontiguousarray(uwpre),
        })
    host = {"u1pre": u1pre, "rsx": rsx, "csp": csp, "csz": csz}
    return in_maps, host


def _make_in_maps(S, Z, X):
    in_maps, host = _host_stats(S, Z, X)
    _CACHED["host"] = host
    return in_maps


def _finale(res):
    """Assemble the scalar objective from device outputs (float64)."""
    host = _CACHED["host"]
    u1pre = host["u1pre"].astype(np.float64)
    rsx = host["rsx"].astype(np.float64)
    csp = host["csp"].astype(np.float64)
    csz = host["csz"].astype(np.float64)

    # u2f is [128, 32] transposed layout: (p, g) -> l = g*128 + p
    u2F = np.asarray(res[0]["u2f"], np.float64).T.ravel()
    h2F = np.concatenate([np.asarray(res[c]["h2f"], np.float64).ravel()
                          for c in range(N_CORES)])
    v2F = np.concatenate([np.asarray(res[c]["v2f"], np.float64).ravel()
                          for c in range(N_CORES)])
    w2F = C3 * F * rsx + np.sum(
        [np.asarray(res[c]["w2p"], np.float64).ravel() for c in range(N_CORES)],
        axis=0)

    u2 = u2F / (F * 2 ** 8)
    w2 = w2F / (F * 2 ** 8)
    h2 = h2F / (F * 2 ** 11)
    v2 = v2F / (F * 2 ** 11)

    rs_sz = u1pre                       # = rsS + b*rsZ
    lR = np.log(R)
    term1 = R * (u2.sum() * h2.sum() + ALPHA * w2.sum() * h2.sum()
                 + BETA * u2.sum() * v2.sum())
    O = (term1
         - (csp.sum() + BETA * csz.sum()) * lR
         - (np.log(u2) * rs_sz).sum()
         - ALPHA * (np.log(w2) * rsx).sum()
         - (np.log(h2) * csp).sum()
         - BETA * (np.log(v2) * csz).sum())
    return np.float32(O)


def _numpy_fallback(S, Z, X, U, H, W, V):
    """Faithful CPU implementation (only used if factors are not all-ones)."""
    S, Z, X, U, H, W, V = [np.asarray(a, np.float32) for a in (S, Z, X, U, H, W, V)]

    def obj(Sp, Xp, Zp):
        return ((Sp - S * np.log(Sp)).sum()
                + ALPHA * (Xp - X * np.log(Xp)).sum()
                + BETA * (Zp - Z * np.log(Zp)).sum())

    Sp = U @ H; Xp = W @ H; Zp = U @ V
    Sd = S / Sp; Xd = X / Xp; Zd = Z / Zp
    O = obj(Sp, Xp, Zp)
    for _ in range(2):
        dHV = H + BETA * V
        U = U * (Sd @ (H / dHV).T + Zd @ ((BETA * V) / dHV).T)
        Sp = U @ H; Zp = U @ V; Sd = S / Sp; Zd = Z / Zp
        dUW = U + ALPHA * W
        H = H * ((U / dUW).T @ Sd + ((ALPHA * W) / dUW).T @ Xd)
        Sp = U @ H; Xp = W @ H; Sd = S / Sp; Xd = X / Xp
        W = W * Xd.sum(axis=1, keepdims=True)
        Xp = W @ H; Xd = X / Xp
        V = V * Zd.sum(axis=0, keepdims=True)
        Zp = U @ V; Zd = Z / Zp
        O = obj(Sp, Xp, Zp)
    return np.float32(O)


def kernel(S, Z, X, U, H, W, V):
    if not (np.all(np.asarray(U) == 1) and np.all(np.asarray(H) == 1)
            and np.all(np.asarray(W) == 1) and np.all(np.asarray(V) == 1)):
        return _numpy_fallback(S, Z, X, U, H, W, V)

    import time
    from concourse.bass_utils import run_bass_kernel_spmd

    if "nc" not in _CACHED:
        _CACHED["nc"] = _build()
    nc = _CACHED["nc"]

    in_maps = _make_in_maps(S, Z, X)
    last = None
    for attempt in range(3):
        try:
            res = run_bass_kernel_spmd(nc, in_maps, core_ids=list(range(N_CORES)))
            return _finale(res.results)
        except Exception as e:  # transient NRT/device errors: reset and retry
            last = e
            try:
                import jax
                jax.clear_caches()
                jax.clear_backends()
            except Exception:
                pass
            time.sleep(3.0)
    raise last


if __name__ == "__main__":
    import reference
    inputs = reference.setup_inputs()
    inputs = {k: np.asarray(v) for k, v in inputs.items()}
    print("kernel:", kernel(**inputs))


# revision 13
# speedup vs baseline: 1.5574x; 1.0654x over previous
"""Trainium2 8-core kernel for nn_ACCSLP_59485297050024 (column-sharded, 1 AllReduce).

The reference is a multiplicative-update NMF-style solver on N=4096 nodes with
rank R=128 and N_ITERS=2, returning a scalar objective O. With all-ones factor
inits the whole computation collapses exactly to rank-1 vector recurrences:

    stage1: h1 = P^T dy1p, v1 = Z^T dy1z          (dy* from host input stats)
    stage2: u2 = Q g1, w1 = X g2, g* = f(h1,v1)   (contraction over columns)
    stage3: h2 = P^T g3, v2 = Z^T g4, g* = f(u2,w1)
    stage4: w2 = X g5, g5 = f(h2)
    O = closed form in (u2, w2, h2, v2, input stats)   [host, float64]

P = S + a*X and Q = S + b*Z (S never appears alone). All matrices stream in
FP8 E4M3; reciprocal stationaries are stored as centered deltas y = c + dy/16
with compile-time centers (validated: objective rel err ~6e-4 vs f32 ref).

Sharding: COLUMN-shard all four matrices (each core owns 512 columns, all
4096 rows). Stages 1 and 3 are then fully LOCAL and only stage 2 needs a
cross-core reduction: one 32KB AllReduce of the (u2, w1) partials. Stage-4
partials are summed on host (8 x 16KB). This leaves a single collective, and
everything after it is arranged to start matmuls as fast as possible:

  - the AR payload is PRE-transposed (PE transposes run in the pre-AR slack),
    so the readback is already partition-major and stage-3 stationaries need
    only vector ops (no PE work) after the AR;
  - the readback DMA runs on the sync queue (the gpsimd queue sits behind a
    ~2us post-collective drain);
  - statP2 is filled before statZ2 so the P stream issues immediately; the
    stage-4 stationary prep overlaps the Z stream via split PSUM banks.
"""

import numpy as np
import ml_dtypes

N = 4096
R = 128
ALPHA = 0.5
BETA = 0.5
N_CORES = 8
CPC = N // N_CORES          # columns per core = 512
NG = N // 128               # row groups = 32 (16 DoubleRow pairs)
NJ8 = N // 512              # 512-wide free chunks = 8

F = 16.0                    # delta-stationary scale
# centered-delta constants (binary-exact in f32); y_scaled = c + dy/F
C1P = 0.96875               # y_s = 2^11/(R(u1+a))
C1Z = 1.0                   # y_s = 2^11/(R u1)
CQ = 1.015625               # y_s = 2^8/(R(h1+b v1))
CX = 1.359375               # y_s = 2^8/(R h1)
C2P = 0.921875              # y_s = 2^11/(R(u2+a w1))
C2Z = 1.328125              # y_s = 2^11/(R u2)
C3 = 1.421875               # y_s = 2^8/(R h2)

_CACHED = {}


def _build():
    import concourse.mybir as mybir
    import concourse.tile as tile
    from concourse import bacc
    from concourse.masks import make_identity

    f8 = mybir.dt.float8e4
    f32 = mybir.dt.float32
    DR = mybir.MatmulPerfMode.DoubleRow
    MUL = mybir.AluOpType.mult
    ADD = mybir.AluOpType.add

    nc = bacc.Bacc("TRN2", target_bir_lowering=False, debug=False,
                   num_devices=N_CORES, dynamic_dma_scratch_size=8192)

    # per-core external inputs (host supplies per-partition-contiguous layouts)
    rp_e = nc.declare_dram_parameter("rp", [128, NG, CPC], f8, isOutput=False)
    rz_e = nc.declare_dram_parameter("rz", [128, NG, CPC], f8, isOutput=False)
    cq_e = nc.declare_dram_parameter("cq", [128, 4, N], f8, isOutput=False)
    cx_e = nc.declare_dram_parameter("cx", [128, 4, N], f8, isOutput=False)
    s1p_e = nc.declare_dram_parameter("s1p", [128, 16, 2, 32], f8, isOutput=False)
    s1z_e = nc.declare_dram_parameter("s1z", [128, 16, 2, 32], f8, isOutput=False)
    cs2_e = nc.declare_dram_parameter("cs2", [2, CPC], f32, isOutput=False)
    cs3p_e = nc.declare_dram_parameter("cs3p", [1, CPC], f32, isOutput=False)
    cs3z_e = nc.declare_dram_parameter("cs3z", [1, CPC], f32, isOutput=False)
    uwpre_e = nc.declare_dram_parameter("uwpre", [128, 64], f32, isOutput=False)
    # per-core external outputs
    out_u2 = nc.declare_dram_parameter("u2f", [128, 32], f32, isOutput=True)
    out_h2 = nc.declare_dram_parameter("h2f", [1, CPC], f32, isOutput=True)
    out_v2 = nc.declare_dram_parameter("v2f", [1, CPC], f32, isOutput=True)
    out_w2p = nc.declare_dram_parameter("w2p", [8, 512], f32, isOutput=True)

    ar_in_t = nc.dram_tensor("ar_in", [128, 64], f32)
    ar_out = nc.dram_tensor("ar_out", [128, 64], f32, addr_space="Shared")
    groups = [list(range(N_CORES))]

    with tile.TileContext(nc) as tc:
        with (
            tc.tile_pool(name="res", bufs=1) as res,
            tc.tile_pool(name="pacc", bufs=1, space="PSUM") as pacc,
            tc.tile_pool(name="ptrans", bufs=1, space="PSUM") as ptrans,
        ):
            # ---------- small inputs ----------
            s1p = res.tile([128, 16, 2, 32], f8, tag="s1p")
            s1z = res.tile([128, 16, 2, 32], f8, tag="s1z")
            cs2 = res.tile([2, CPC], f32, tag="cs2")
            cs3p = res.tile([1, CPC], f32, tag="cs3p")
            cs3z = res.tile([1, CPC], f32, tag="cs3z")
            uwpre = res.tile([128, 64], f32, tag="uwpre")
            for t, e in ((s1p, s1p_e), (s1z, s1z_e), (cs2, cs2_e),
                         (cs3p, cs3p_e), (cs3z, cs3z_e), (uwpre, uwpre_e)):
                nc.sync.dma_start(t[:], e[:])

            ident = res.tile([128, 128], f32, tag="ident")
            make_identity(nc, ident[:])

            # device-filled fp8 stationaries (values land in one column per
            # variant; zero-fill once, off critical path)
            statQ = res.tile([128, 2, NJ8, 2, 32], f8, tag="statQ")
            statX = res.tile([128, 2, NJ8, 2, 32], f8, tag="statX")
            statP2 = res.tile([128, 16, 2, 32], f8, tag="statP2")
            statZ2 = res.tile([128, 16, 2, 32], f8, tag="statZ2")
            stat3 = res.tile([128, 2, NJ8, 2, 32], f8, tag="stat3")
            for t in (statQ, statX, statP2, statZ2, stat3):
                nc.gpsimd.memset(t[:], 0.0)

            # ---------- resident loads (pieces, ordered for pipelining) ----
            tRp = [res.tile([128, 8, CPC], f8, name=f"tRp{q}", tag=f"tRp{q}") for q in range(4)]
            tRz = [res.tile([128, 16, CPC], f8, name=f"tRz{q}", tag=f"tRz{q}") for q in range(2)]
            tCq = [res.tile([128, 4, 2048], f8, name=f"tCq{q}", tag=f"tCq{q}") for q in range(2)]
            tCx = [res.tile([128, 4, 2048], f8, name=f"tCx{q}", tag=f"tCx{q}") for q in range(2)]
            for q in range(4):
                nc.sync.dma_start(tRp[q][:], rp_e[:, 8 * q:8 * q + 8, :])
            for q in range(2):
                nc.sync.dma_start(tRz[q][:], rz_e[:, 16 * q:16 * q + 16, :])
            for q in range(2):
                nc.sync.dma_start(tCq[q][:], cq_e[:, :, 2048 * q:2048 * q + 2048])
            for q in range(2):
                nc.sync.dma_start(tCx[q][:], cx_e[:, :, 2048 * q:2048 * q + 2048])

            def movRp(i):
                return tRp[i // 4][:, (i % 4) * 2:(i % 4) * 2 + 2, :]

            def movRz(i):
                return tRz[i // 8][:, (i % 8) * 2:(i % 8) * 2 + 2, :]

            def movC(pieces, c8, i):
                w = (c8 % 4) * 512
                return pieces[c8 // 4][:, 2 * i:2 * i + 2, w:w + 512]

            # ---------- stage 1: h1, v1 (local col slices) ----------
            psS1 = pacc.tile([32, CPC], f32, tag="psS1")
            for i in range(16):
                nc.tensor.matmul(psS1[:], s1p[:, i, :, :], movRp(i),
                                 start=(i == 0), stop=False, perf_mode=DR)
            for i in range(16):
                nc.tensor.matmul(psS1[:], s1z[:, i, :, :], movRz(i),
                                 start=False, stop=(i == 15), perf_mode=DR)
            # rows: 0 = dP (h1 part), 1 = dZ (v1 part)
            s1out = res.tile([2, CPC], f32, tag="s1out")
            nc.scalar.copy(s1out[:], psS1[0:2, :])
            # h1F = F*C1P*csp + dP ; v1F = F*C1Z*csz + dZ   (cs2 host-prescaled)
            h1v1 = res.tile([2, CPC], f32, tag="h1v1")
            nc.vector.scalar_tensor_tensor(h1v1[:], cs2[:], F, s1out[:], MUL, ADD)
            # transpose (PE crosses partitions): ps_t cols k+4r, r=0 -> h1F by
            # group g=k (cols 0:4), r=1 -> v1F (cols 4:8)
            ps_t2 = ptrans.tile([128, 64], f32, tag="pt")
            for k in range(4):
                nc.tensor.transpose(ps_t2[:, k:8:4], h1v1[:, 128 * k:128 * (k + 1)],
                                    ident[0:2, 0:2])
            tp2 = res.tile([128, 8], f32, tag="tp2")
            nc.vector.tensor_copy(tp2[:], ps_t2[:, 0:8])
            pre_a = res.tile([128, 8], f32, tag="pre_a")
            # cols 0:4 = 1/(h1F + b*v1F), cols 4:8 = 1/h1F
            tq2t = res.tile([128, 4], f32, tag="tq2t")
            nc.vector.scalar_tensor_tensor(tq2t[:], tp2[:, 4:8], BETA, tp2[:, 0:4],
                                           MUL, ADD)
            nc.vector.reciprocal(pre_a[:, 0:4], tq2t[:])
            nc.vector.reciprocal(pre_a[:, 4:8], tp2[:, 0:4])
            # statQ: y-values in column c8 (-> psum row c8); statX at c8+8
            for c8 in range(NJ8):
                nc.vector.tensor_scalar(
                    statQ[:, :, c8, :, c8],
                    pre_a[:, 0:4].rearrange("p (c j) -> p c j", j=2),
                    float(2 ** 20), -F * CQ, MUL, ADD)
                nc.vector.tensor_scalar(
                    statX[:, :, c8, :, c8 + 8],
                    pre_a[:, 4:8].rearrange("p (c j) -> p c j", j=2),
                    float(2 ** 20), -F * CX, MUL, ADD)

            # ---------- stage 2: u2, w1 partials ----------
            psQX = pacc.tile([32, 512], f32, tag="psQX")
            for c8 in range(NJ8):
                for i in range(2):
                    nc.tensor.matmul(psQX[:], statQ[:, i, c8, :, :], movC(tCq, c8, i),
                                     start=(c8 == 0 and i == 0), stop=False,
                                     perf_mode=DR)
            for c8 in range(NJ8):
                for i in range(2):
                    nc.tensor.matmul(psQX[:], statX[:, i, c8, :, :], movC(tCx, c8, i),
                                     start=False, stop=(c8 == NJ8 - 1 and i == 1),
                                     perf_mode=DR)
            # pre-transpose the AR payload (pre-AR slack): rows 0-7 = u2
            # partial chunks, 8-15 = w1 -> [128, 64] partition-major
            stQX = res.tile([16, 512], f32, tag="stQX")
            nc.scalar.copy(stQX[:], psQX[0:16, :])
            ps_tq = ptrans.tile([128, 64], f32, tag="pt")
            for k in range(4):
                nc.tensor.transpose(ps_tq[:, k:64:4], stQX[:, 128 * k:128 * (k + 1)],
                                    ident[0:16, 0:16])
            arr = res.tile([128, 64], f32, tag="arr")
            nc.vector.tensor_copy(arr[:], ps_tq[:])
            nc.gpsimd.dma_start(ar_in_t[:], arr[:])
            nc.gpsimd.collective_compute(
                "AllReduce", mybir.AluOpType.add, replica_groups=groups,
                ins=[ar_in_t[:].opt()], outs=[ar_out[:].opt()])

            # ---------- stage 3 stationaries from AR (vector-only) ----------
            ar_rd = res.tile([128, 64], f32, tag="ar_rd")
            nc.sync.dma_start(ar_rd[:], ar_out[:])
            # cols 0:32: u2F = F*CQ*u1pre + arQ ; 32:64: w1F = F*CX*rsx + arX
            uw = res.tile([128, 64], f32, tag="uw")
            nc.vector.scalar_tensor_tensor(uw[:], uwpre[:], F, ar_rd[:], MUL, ADD)
            t2t = res.tile([128, 32], f32, tag="t2t")
            nc.vector.scalar_tensor_tensor(t2t[:], uw[:, 32:64], ALPHA, uw[:, 0:32],
                                           MUL, ADD)
            pre_b = res.tile([128, 64], f32, tag="pre_b")
            nc.vector.reciprocal(pre_b[:, 0:32], t2t[:])
            nc.vector.tensor_scalar(
                statP2[:, :, :, 0],
                pre_b[:, 0:32].rearrange("p (c j) -> p c j", j=2),
                float(2 ** 20), -F * C2P, MUL, ADD)
            nc.sync.dma_start(out_u2[:], uw[:, 0:32])

            # ---------- stage 3: h2 (P stream starts asap) ----------
            psS3a = pacc.tile([32, CPC], f32, tag="psS3a")
            for i in range(16):
                nc.tensor.matmul(psS3a[:], statP2[:, i, :, :], movRp(i),
                                 start=(i == 0), stop=(i == 15), perf_mode=DR)
            # v2 stationaries fill during the P stream
            nc.vector.reciprocal(pre_b[:, 32:64], uw[:, 0:32])
            nc.vector.tensor_scalar(
                statZ2[:, :, :, 0],
                pre_b[:, 32:64].rearrange("p (c j) -> p c j", j=2),
                float(2 ** 20), -F * C2Z, MUL, ADD)
            # h2F = F*C2P*csp + dP2 ; stage-4 prep overlaps the Z stream below
            h2f = res.tile([1, CPC], f32, tag="h2f")
            nc.vector.scalar_tensor_tensor(h2f[:], cs3p[:], F, psS3a[0:1, :], MUL, ADD)
            nc.sync.dma_start(out_h2[:], h2f[:])

            # stage-4 transposes right after h2f (Z stream hides the recip+fills)
            ps_t4 = ptrans.tile([128, 64], f32, tag="pt")
            for k in range(4):
                nc.tensor.transpose(ps_t4[:, k:k + 1], h2f[:, 128 * k:128 * (k + 1)],
                                    ident[0:1, 0:1])

            # ---------- stage 3b: v2 (separate bank) ----------
            psS3b = pacc.tile([32, CPC], f32, tag="psS3b")
            for i in range(16):
                nc.tensor.matmul(psS3b[:], statZ2[:, i, :, :], movRz(i),
                                 start=(i == 0), stop=(i == 15), perf_mode=DR)

            # ---------- stage 4: w2 partial ----------
            pre_3 = res.tile([128, 4], f32, tag="pre_3")
            nc.vector.reciprocal(pre_3[:], ps_t4[:, 0:4])
            for c8 in range(NJ8):
                nc.vector.tensor_scalar(
                    stat3[:, :, c8, :, c8],
                    pre_3[:, 0:4].rearrange("p (c j) -> p c j", j=2),
                    float(2 ** 20), -F * C3, MUL, ADD)
            # v2 output (vector op after the stat3 fills so it can't stall them)
            v2f = res.tile([1, CPC], f32, tag="v2f")
            nc.vector.scalar_tensor_tensor(v2f[:], cs3z[:], F, psS3b[0:1, :], MUL, ADD)
            nc.sync.dma_start(out_v2[:], v2f[:])
            psX4 = pacc.tile([32, 512], f32, tag="psX4")
            for c8 in range(NJ8):
                for i in range(2):
                    nc.tensor.matmul(psX4[:], stat3[:, i, c8, :, :], movC(tCx, c8, i),
                                     start=(c8 == 0 and i == 0),
                                     stop=(c8 == NJ8 - 1 and i == 1), perf_mode=DR)
            stW = res.tile([8, 512], f32, tag="stW")
            nc.scalar.copy(stW[:], psX4[0:8, :])
            nc.sync.dma_start(out_w2p[:], stW[:])

    nc.compile()
    return nc


def _host_stats(S, Z, X):
    """fp8 casts + input statistics; returns per-core in_maps and host data."""
    S = np.asarray(S, np.float32)
    Z = np.asarray(Z, np.float32)
    X = np.asarray(X, np.float32)
    P8 = (S + ALPHA * X).astype(ml_dtypes.float8_e4m3)
    Q8 = (S + BETA * Z).astype(ml_dtypes.float8_e4m3)
    X8 = X.astype(ml_dtypes.float8_e4m3)
    Z8 = Z.astype(ml_dtypes.float8_e4m3)

    Pf = P8.astype(np.float32)
    Qf = Q8.astype(np.float32)
    Xf = X8.astype(np.float32)
    Zf = Z8.astype(np.float32)
    u1pre = Qf.sum(axis=1)                  # rowsum(Q)  (N,)
    rsx = Xf.sum(axis=1)                    # rowsum(X)
    csp = Pf.sum(axis=0)                    # colsum(P)  (N,)
    csz = Zf.sum(axis=0)

    qq = u1pre * (2.0 / 3.0)                # R*u1
    dy1p = ((2048.0 / (qq + 64.0)) - C1P) * F
    dy1z = ((2048.0 / qq) - C1Z) * F

    def stat_layout(v, col):
        # [4096] -> [128(p), 16(pair), 2(k), 32(col)], values at `col`
        g = v.reshape(NG, 128).T                 # [128, 32]; l = g*128 + p
        out = np.zeros((128, 16, 2, 32), v.dtype)
        out[:, :, 0, col] = g[:, 0::2]
        out[:, :, 1, col] = g[:, 1::2]
        return out

    s1p = stat_layout(dy1p.astype(ml_dtypes.float8_e4m3), 0)
    s1z = stat_layout(dy1z.astype(ml_dtypes.float8_e4m3), 1)

    def row_layout(colshard):
        # [4096, 512] -> [128(p), NG(g), 512(j)]; row l = g*128 + p
        return np.ascontiguousarray(
            colshard.reshape(NG, 128, CPC).transpose(1, 0, 2))

    def col_layout(colshard):
        # [4096, 512] -> [128(p), 4(a), 4096(l)]; col j_local = a*128 + p
        return np.ascontiguousarray(
            colshard.T.reshape(4, 128, N).transpose(1, 0, 2))

    # [128, 64]: cols 0:32 = CQ*u1pre, 32:64 = CX*rsx, both at (p,g)=l=g*128+p
    uwpre = np.concatenate(
        [CQ * u1pre.reshape(32, 128).T, CX * rsx.reshape(32, 128).T],
        axis=1).astype(np.float32)

    in_maps = []
    for c in range(N_CORES):
        cols = slice(c * CPC, (c + 1) * CPC)
        cs2 = np.stack([C1P * csp[cols], C1Z * csz[cols]]).astype(np.float32)
        in_maps.append({
            "rp": row_layout(P8[:, cols]), "rz": row_layout(Z8[:, cols]),
            "cq": col_layout(Q8[:, cols]), "cx": col_layout(X8[:, cols]),
            "s1p": s1p, "s1z": s1z,
            "cs2": np.ascontiguousarray(cs2),
            "cs3p": np.ascontiguousarray(C2P * csp[cols]).reshape(1, CPC).astype(np.float32),
            "cs3z": np.ascontiguousarray(C2Z * csz[cols]).reshape(1, CPC).astype(np.float32),
            "uwpre": np.ascontiguousarray(uwpre),
        })
    host = {"u1pre": u1pre, "rsx": rsx, "csp": csp, "csz": csz}
    return in_maps, host


def _make_in_maps(S, Z, X):
    in_maps, host = _host_stats(S, Z, X)
    _CACHED["host"] = host
    return in_maps


def _finale(res):
    """Assemble the scalar objective from device outputs (float64)."""
    host = _CACHED["host"]
    u1pre = host["u1pre"].astype(np.float64)
    rsx = host["rsx"].astype(np.float64)
    csp = host["csp"].astype(np.float64)
    csz = host["csz"].astype(np.float64)

    # u2f is [128, 32] transposed layout: (p, g) -> l = g*128 + p
    u2F = np.asarray(res[0]["u2f"], np.float64).T.ravel()
    h2F = np.concatenate([np.asarray(res[c]["h2f"], np.float64).ravel()
                          for c in range(N_CORES)])
    v2F = np.concatenate([np.asarray(res[c]["v2f"], np.float64).ravel()
                          for c in range(N_CORES)])
    w2F = C3 * F * rsx + np.sum(
        [np.asarray(res[c]["w2p"], np.float64).ravel() for c in range(N_CORES)],
        axis=0)

    u2 = u2F / (F * 2 ** 8)
    w2 = w2F / (F * 2 ** 8)
    h2 = h2F / (F * 2 ** 11)
    v2 = v2F / (F * 2 ** 11)

    rs_sz = u1pre                       # = rsS + b*rsZ
    lR = np.log(R)
    term1 = R * (u2.sum() * h2.sum() + ALPHA * w2.sum() * h2.sum()
                 + BETA * u2.sum() * v2.sum())
    O = (term1
         - (csp.sum() + BETA * csz.sum()) * lR
         - (np.log(u2) * rs_sz).sum()
         - ALPHA * (np.log(w2) * rsx).sum()
         - (np.log(h2) * csp).sum()
         - BETA * (np.log(v2) * csz).sum())
    return np.float32(O)


def _numpy_fallback(S, Z, X, U, H, W, V):
    """Faithful CPU implementation (only used if factors are not all-ones)."""
    S, Z, X, U, H, W, V = [np.asarray(a, np.float32) for a in (S, Z, X, U, H, W, V)]

    def obj(Sp, Xp, Zp):
        return ((Sp - S * np.log(Sp)).sum()
                + ALPHA * (Xp - X * np.log(Xp)).sum()
                + BETA * (Zp - Z * np.log(Zp)).sum())

    Sp = U @ H; Xp = W @ H; Zp = U @ V
    Sd = S / Sp; Xd = X / Xp; Zd = Z / Zp
    O = obj(Sp, Xp, Zp)
    for _ in range(2):
        dHV = H + BETA * V
        U = U * (Sd @ (H / dHV).T + Zd @ ((BETA * V) / dHV).T)
        Sp = U @ H; Zp = U @ V; Sd = S / Sp; Zd = Z / Zp
        dUW = U + ALPHA * W
        H = H * ((U / dUW).T @ Sd + ((ALPHA * W) / dUW).T @ Xd)
        Sp = U @ H; Xp = W @ H; Sd = S / Sp; Xd = X / Xp
        W = W * Xd.sum(axis=1, keepdims=True)
        Xp = W @ H; Xd = X / Xp
        V = V * Zd.sum(axis=0, keepdims=True)
        Zp = U @ V; Zd = Z / Zp
        O = obj(Sp, Xp, Zp)
    return np.float32(O)


def kernel(S, Z, X, U, H, W, V):
    if not (np.all(np.asarray(U) == 1) and np.all(np.asarray(H) == 1)
            and np.all(np.asarray(W) == 1) and np.all(np.asarray(V) == 1)):
        return _numpy_fallback(S, Z, X, U, H, W, V)

    import time
    from concourse.bass_utils import run_bass_kernel_spmd

    if "nc" not in _CACHED:
        _CACHED["nc"] = _build()
    nc = _CACHED["nc"]

    in_maps = _make_in_maps(S, Z, X)
    last = None
    for attempt in range(3):
        try:
            res = run_bass_kernel_spmd(nc, in_maps, core_ids=list(range(N_CORES)))
            return _finale(res.results)
        except Exception as e:  # transient NRT/device errors: reset and retry
            last = e
            try:
                import jax
                jax.clear_caches()
                jax.clear_backends()
            except Exception:
                pass
            time.sleep(3.0)
    raise last


if __name__ == "__main__":
    import reference
    inputs = reference.setup_inputs()
    inputs = {k: np.asarray(v) for k, v in inputs.items()}
    print("kernel:", kernel(**inputs))
